# revision 42
# baseline (speedup 1.0000x reference)
"""Trainium2 Bass kernel for DiffGeomPropsApprox (within-batch uv-space 16-NN
-> neighborhood covariance of X -> descending symmetric-3x3 eigenvalues).

Sharding: data-parallel over batch B=8, one batch per NeuronCore (8 cores).

Spatial windowing: host-side, each batch's points are sorted into a
serpentine strip order (strips = v-rank quantiles, u alternating
direction). A conservative per-query radius bound R(q) - refined to the
16th-smallest distance within a provably-sufficient cell box - yields,
per 128-query tile, a contiguous sorted-index window that contains every
query's exact 16-NN. The device only computes distances/top-k/mask/matmul
over that window (~5-7 of 32 tiles). Windows are computed at runtime from
the actual inputs (union across batches; one SPMD program).

Per-query-tile device pipeline (software-pipelined stage emission with
skews squares@0 / neg@+1 / sel@+3 / mask@+4 / matmul@+5 / fixup@+6, each
engine's in-order queue only sees work whose producers ran earlier):
  ACT:   squ=(u_c-u_q)^2, sqv=(v_c-v_q)^2 (Square, per-partition bias;
         exact reference f32 rounding)
  neg:   negdm = -(squ+sqv) exactly; production rotates GPSIMD-double /
         DVE fused custom op (LN_BWD_DX_ANT) / ACT-neg+GPSIMD-sub
  DVE:   max8 -> match_replace -> max8 over negdm -> -d16 (16th smallest)
  ACT:   mask = Sign(negdm + d16*(1+2^-22)) in {-1,+1} bf16
  DMA:   x-bar transpose mask -> [c-part, q]
  PE:    per window tile j: matmul acc += fbf_j^T wmask_j (18 features =
         9 x bf16 hi/lo) and ftot += fbf_j^T ones (separate PSUM groups)
  ACT:   fixup sum_sel = (acc+ftot)/2 via Identity bias-add; PE transpose;
         DVE slab copy.
Startup: partition-broadcast of u/v rows via PE fp32 ones-outer-product
(exact), PSUM-chunked; all input DMAs contiguous (host pre-transposes).
Eigenvalues: closed-form trig method in 3 chunks; sqrt phase (A) overlaps
the main loop via generator-spread emission, arctan/sin phase (B) runs
once at the end grouped by ACT table-set to avoid LUT reload thrash.
"""

from contextlib import ExitStack

import numpy as np

import concourse.bass as bass
import concourse.tile as tile
from concourse import bacc, mybir
from concourse.alu_op_type import AluOpType
from concourse.bass_utils import run_bass_kernel_spmd
from concourse.dve_ops import LN_BWD_DX_ANT

F32 = mybir.dt.float32
BF16 = mybir.dt.bfloat16
I32 = mybir.dt.int32
AF = mybir.ActivationFunctionType
OP = AluOpType

P = 128
K = 16
NEG_BIG = -3.0e38
PI = float(np.pi)
EPS_REL = 1.0 + 2.0 ** -22

G_STRIPS = 32          # equal-count strips (by v-rank)
H_INV = 256            # grid resolution for the coarse R(q) bound
H2_INV = 32            # cell-list resolution for the refined bound


# --------------------------- host-side windowing --------------------------- #

def _strip_perm(uv, G=G_STRIPS):
    """Serpentine strip order: strip = v-rank quantile, u asc/desc."""
    M = uv.shape[0]
    u, v = uv[:, 0], uv[:, 1]
    rank_v = np.empty(M, np.int64)
    rank_v[np.argsort(v, kind="stable")] = np.arange(M)
    strip = rank_v * G // M
    ukey = np.where(strip % 2 == 0, u, -u)
    return np.lexsort((ukey, strip))


def _geom_R(uv, h_inv=H_INV):
    """Coarse conservative bound on the 16-NN radius: smallest (2r+1)^2
    cell box centered on the point's cell holding >= K points; any K
    points in that box lie within (r+1)*h*sqrt(2)."""
    M = uv.shape[0]
    u, v = uv[:, 0], uv[:, 1]
    ci = np.minimum((u * h_inv).astype(np.int64), h_inv - 1)
    cj = np.minimum((v * h_inv).astype(np.int64), h_inv - 1)
    H = np.zeros((h_inv, h_inv), np.int64)
    np.add.at(H, (ci, cj), 1)
    S = np.zeros((h_inv + 1, h_inv + 1), np.int64)
    S[1:, 1:] = H.cumsum(0).cumsum(1)
    R = np.full(M, np.sqrt(2.0))
    done = np.zeros(M, bool)
    for rho in range(1, h_inv):
        i0 = np.clip(ci - rho, 0, h_inv); i1 = np.clip(ci + rho + 1, 0, h_inv)
        j0 = np.clip(cj - rho, 0, h_inv); j1 = np.clip(cj + rho + 1, 0, h_inv)
        cnt = S[i1, j1] - S[i0, j1] - S[i1, j0] + S[i0, j0]
        new = (~done) & (cnt >= K)
        R[new] = (rho + 1) / h_inv * np.sqrt(2.0)
        done |= new
        if done.all():
            break
    return R


def _refine_R(uv, R0, h_inv=H2_INV):
    """Tighten R to the 16th-smallest distance within the cell box that
    provably covers disc(q, R0).  disc(q, R0) holds >= K points (by R0's
    construction), the box covers the disc, so the 16th-nearest within
    the box is <= R0 and >= the true 16-NN radius."""
    M = uv.shape[0]
    u, v = uv[:, 0].astype(np.float64), uv[:, 1].astype(np.float64)
    ci = np.minimum((u * h_inv).astype(np.int64), h_inv - 1)
    cj = np.minimum((v * h_inv).astype(np.int64), h_inv - 1)
    cell = ci * h_inv + cj
    order = np.argsort(cell, kind="stable")
    csort = cell[order]
    ncell = h_inv * h_inv
    starts = np.searchsorted(csort, np.arange(ncell + 1))
    cnts = np.diff(starts)
    cmax = int(cnts.max())
    C = np.full((ncell, cmax), -1, np.int64)
    for c in range(ncell):
        C[c, :cnts[c]] = order[starts[c]:starts[c + 1]]
    rho = np.ceil(R0 * h_inv).astype(np.int64)
    R1 = np.empty(M, np.float64)
    for rv in np.unique(rho):
        sel = np.where(rho == rv)[0]
        offs = [(di, dj) for di in range(-rv, rv + 1)
                for dj in range(-rv, rv + 1)]
        cand = np.empty((len(sel), len(offs) * cmax), np.int64)
        for k, (di, dj) in enumerate(offs):
            ii = np.clip(ci[sel] + di, 0, h_inv - 1)
            jj = np.clip(cj[sel] + dj, 0, h_inv - 1)
            # out-of-range clamps may duplicate cells; harmless (extra
            # candidates only shrink the 16th distance toward truth; they
            # are real points so the bound stays valid)
            cand[:, k * cmax:(k + 1) * cmax] = C[ii * h_inv + jj]
        pad = cand < 0
        cid = np.where(pad, 0, cand)
        d2 = (u[sel, None] - u[cid]) ** 2 + (v[sel, None] - v[cid]) ** 2
        d2[pad] = np.inf
        # dedupe isn't needed for validity, but clamp-duplicated points
        # could make the 16th-smallest too small -> not conservative.
        # Sort candidate ids per row and inf-out repeats.
        si = np.argsort(cand, axis=1, kind="stable")
        cs = np.take_along_axis(cand, si, 1)
        dup = np.zeros_like(pad)
        dup[:, 1:] = cs[:, 1:] == cs[:, :-1]
        ds = np.take_along_axis(d2, si, 1)
        ds[dup | (cs < 0)] = np.inf
        R1[sel] = np.sqrt(np.partition(ds, K - 1, axis=1)[:, K - 1])
    return np.minimum(R0, np.nextafter(R1.astype(np.float32),
                                       np.float32(np.inf)))


def _batch_windows(uv, G=G_STRIPS):
    """perm + per-tile [jlo, jhi] (tile units) windows containing, for
    every query in the tile, all points within R(q) (hence its 16-NN)."""
    M = uv.shape[0]
    T = M // P
    spts = M // G
    perm = _strip_perm(uv, G)
    us, vs = uv[perm, 0], uv[perm, 1]
    R = _refine_R(uv, _geom_R(uv))
    Rq = R[perm]
    vmin = vs.reshape(G, spts).min(1)
    vmax = vs.reshape(G, spts).max(1)
    slo = np.clip(np.searchsorted(vmax, vs - Rq, side="left"), 0, G - 1)
    shi = np.clip(np.searchsorted(vmin, vs + Rq, side="right") - 1, 0, G - 1)
    plo = np.empty(M, np.int64)
    phi = np.empty(M, np.int64)
    for s in range(G):
        base = s * spts
        su = us[base:base + spts]
        asc = (s % 2 == 0)
        sua = su if asc else su[::-1]
        for sel, is_lo in ((slo == s, True), (shi == s, False)):
            if not sel.any():
                continue
            ulo = us[sel] - Rq[sel]
            uhi = us[sel] + Rq[sel]
            if is_lo:
                off = (np.searchsorted(sua, ulo, side="left") if asc else
                       spts - np.searchsorted(sua, uhi, side="right"))
                plo[sel] = base + np.clip(off, 0, spts - 1)
            else:
                off = (np.searchsorted(sua, uhi, side="right") - 1 if asc else
                       spts - 1 - np.searchsorted(sua, ulo, side="left"))
                phi[sel] = base + np.clip(off, 0, spts - 1)
    phi = np.maximum(phi, plo)
    jlo = np.empty(T, np.int64)
    jhi = np.empty(T, np.int64)
    c0e = np.empty(T, np.int64)
    c1e = np.empty(T, np.int64)
    for t in range(T):
        c0e[t] = plo[t * P:(t + 1) * P].min()
        c1e[t] = phi[t * P:(t + 1) * P].max() + 1
        jlo[t] = c0e[t] // P
        jhi[t] = (c1e[t] - 1) // P
    return perm, jlo, jhi, c0e, c1e


# ----------------------------- device kernel ------------------------------- #

def _emit(ctx, tc, out_ap, uvt_ap, uvq_ap, xq_ap, M, ident18, jlo, jhi,
          c0e, c1e):
    nc = tc.nc
    T = M // P
    NF = 18
    WTS = [int(jhi[t] - jlo[t] + 1) for t in range(T)]
    WMAX = max(WTS) * P
    # column-exact sub-ranges within each tile window (8-aligned) for the
    # pointwise passes; the pad columns only exist in the mask (set to -1)
    A0 = [(int(c0e[t]) - int(jlo[t]) * P) // 8 * 8 for t in range(T)]
    A1 = [min(-(-(int(c1e[t]) - int(jlo[t]) * P) // 8) * 8, WTS[t] * P)
          for t in range(T)]

    const = ctx.enter_context(tc.tile_pool(name="const", bufs=1))
    work = ctx.enter_context(tc.tile_pool(name="work", bufs=2))
    small = ctx.enter_context(tc.tile_pool(name="small", bufs=14))
    psum = ctx.enter_context(tc.tile_pool(name="psum", bufs=2, space="PSUM"))
    epool = ctx.enter_context(tc.tile_pool(name="eig", bufs=1))

    # ---- broadcast candidate coords across partitions ----
    # uvt is host-transposed [2, M]: step-0 loads are contiguous rows.
    # Doubling chain, column-split across two queues per coordinate.
    # slab layouts [P, T, k]: host pre-transposed so loads are contiguous
    uv_slab = const.tile([P, T, 2], F32, tag="uv_slab")
    uv_r = uvq_ap.rearrange("(p t) k -> p t k", p=P)
    x_slab = const.tile([P, T, 3], F32, tag="x_slab")
    x_r = xq_ap.rearrange("(p t) k -> p t k", p=P)
    nc.sync.dma_start(uv_slab[:], uv_r[:])
    nc.scalar.dma_start(x_slab[:], x_r[:])

    # partition-broadcast of u/v rows via PE fp32 outer product with a ones
    # column (exact: 1.0*x). 512-col PSUM chunks, copied out on ACT/DVE.
    u_b = const.tile([P, M], F32, tag="u_b")
    v_b = const.tile([P, M], F32, tag="v_b")
    ones1 = const.tile([1, P], F32, tag="ones1")
    nc.gpsimd.memset(ones1[:], 1.0)
    urow = const.tile([1, M], F32, tag="urow")
    vrow = const.tile([1, M], F32, tag="vrow")
    nc.sync.dma_start(urow[:], uvt_ap[0:1, :])
    nc.scalar.dma_start(vrow[:], uvt_ap[1:2, :])
    BC = 512
    for ci, c in enumerate(range(0, M, BC)):
        for row, dst in ((urow, u_b), (vrow, v_b)):
            pb = psum.tile([P, BC], F32, tag="bc", name="bc", bufs=2)
            nc.tensor.matmul(pb[:], lhsT=ones1[:], rhs=row[0:1, c:c + BC],
                             start=True, stop=True)
            if (ci + (0 if dst is u_b else 1)) % 2 == 0:
                nc.vector.tensor_copy(dst[:, c:c + BC], pb[:])
            else:
                nc.scalar.copy(dst[:, c:c + BC], pb[:])
    nuv = const.tile([P, T, 2], F32, tag="nuv")
    nc.vector.tensor_scalar(out=nuv[:], in0=uv_slab[:], scalar1=-1.0,
                            scalar2=None, op0=OP.mult)

    # ---- features: [x y z x2 y2 z2 xy xz yz] as bf16 hi/lo ----
    pairs = [(0, 0), (1, 1), (2, 2), (0, 1), (0, 2), (1, 2)]
    fsl = work.tile([P, T, 9], F32, tag="fsl", name="fsl", bufs=1)
    nc.vector.tensor_copy(fsl[:, :, 0:3], x_slab[:])
    for i, (a, b) in enumerate(pairs):
        nc.vector.tensor_tensor(out=fsl[:, :, 3 + i], in0=x_slab[:, :, a],
                                in1=x_slab[:, :, b], op=OP.mult)
    fbf = const.tile([P, T, NF], BF16, tag="fbf")
    nc.vector.tensor_copy(fbf[:, :, 0:9], fsl[:])
    fhi32 = work.tile([P, T, 9], F32, tag="fhi32", name="fhi32", bufs=1)
    nc.vector.tensor_copy(fhi32[:], fbf[:, :, 0:9])
    nc.vector.tensor_tensor(out=fbf[:, :, 9:18], in0=fsl[:], in1=fhi32[:],
                            op=OP.subtract)

    ones_c = const.tile([P, 1], BF16, tag="ones_c")
    nc.gpsimd.memset(ones_c[:], 1.0)
    zeros = const.tile([P, WMAX], F32, tag="zeros")
    nc.gpsimd.memset(zeros[:], 0.0)
    bias_c = const.tile([P, 2], F32, tag="bias_c")
    nc.gpsimd.memset(bias_c[:, 0:1], PI / 2)
    nc.gpsimd.memset(bias_c[:, 1:2], PI / 6)

    cov = const.tile([P, T, NF], F32, tag="cov")

    # ---- pipeline stages -------------------------------------------------
    state = {}

    def st_squares(t):
        c0 = int(jlo[t]) * P
        a0, a1 = A0[t], A1[t]
        squ = work.tile([P, WMAX], F32, tag="sq", name="squ", bufs=8)
        nc.scalar.activation(squ[:, a0:a1], u_b[:, c0 + a0:c0 + a1],
                             AF.Square, bias=nuv[:, t, 0:1], scale=1.0)
        sqv = work.tile([P, WMAX], F32, tag="sq", name="sqv", bufs=8)
        nc.scalar.activation(sqv[:, a0:a1], v_b[:, c0 + a0:c0 + a1],
                             AF.Square, bias=nuv[:, t, 1:2], scale=1.0)
        state[t] = {"squ": squ, "sqv": sqv, "w": WTS[t] * P}

    def st_neg(t):
        # negdm = -(squ + sqv), exact; production rotates across engines:
        #   t%3==0: GPSIMD (0-squ) then (nsq-sqv)
        #   t%3==1: DVE fused custom op (squ - sqv*-1 - 0) * -1
        #   t%3==2: ACT -squ, then GPSIMD (nsq - sqv)
        s = state[t]
        a0, a1 = A0[t], A1[t]
        negdm = work.tile([P, WMAX], F32, tag="negdm", name="negdm", bufs=6)
        if t % 3 == 1:
            nc.vector._custom_dve(LN_BWD_DX_ANT, out=negdm[:, a0:a1],
                                  in0=s["squ"][:, a0:a1],
                                  in1=s["sqv"][:, a0:a1],
                                  s0=-1.0, s1=0.0, imm2=-1.0)
        else:
            nsq = work.tile([P, WMAX], F32, tag="dm", name="nsq", bufs=4)
            nc.gpsimd.tensor_tensor(out=nsq[:, a0:a1],
                                    in0=zeros[:, a0:a1],
                                    in1=s["squ"][:, a0:a1],
                                    op=OP.subtract)
            nc.gpsimd.tensor_tensor(out=negdm[:, a0:a1], in0=nsq[:, a0:a1],
                                    in1=s["sqv"][:, a0:a1], op=OP.subtract)
        s["negdm"] = negdm

    def st_sel(t):
        s = state[t]
        a0, a1 = A0[t], A1[t]
        negdm = s["negdm"]
        m1 = small.tile([P, 8], F32, tag="m1", name="m1")
        nc.vector.max(m1[:], negdm[:, a0:a1])
        mr = work.tile([P, WMAX], F32, tag="mr", name="mr", bufs=3)
        nc.vector.match_replace(mr[:, a0:a1], m1[:], negdm[:, a0:a1],
                                NEG_BIG)
        m2 = small.tile([P, 8], F32, tag="m2", name="m2")
        nc.vector.max(m2[:], mr[:, a0:a1])
        # Sign-mask bias: +d16*(1+2^-22)  (m2[7] = -d16)
        nt16p = small.tile([P, 1], F32, tag="nt16p", name="nt16p")
        nc.vector.tensor_scalar(out=nt16p[:], in0=m2[:, 7:8],
                                scalar1=-EPS_REL, scalar2=None, op0=OP.mult)
        s["nt16p"] = nt16p

    def st_mask(t):
        s = state[t]
        w = s["w"]
        a0, a1 = A0[t], A1[t]
        # {-1,+1} mask in bf16 on ACT (Sign LUT); +1 iff d <= d16*(1+eps).
        # Pad columns outside [a0,a1) hold -1 (unselected) so the ftot
        # correction stays consistent over the full tile window.
        wmask = work.tile([P, WMAX], BF16, tag="wmask", name="wmask", bufs=3)
        if a0 > 0:
            nc.vector.memset(wmask[:, 0:a0], -1.0)
        if a1 < w:
            nc.vector.memset(wmask[:, a1:w], -1.0)
        nc.scalar.activation(wmask[:, a0:a1], s["negdm"][:, a0:a1], AF.Sign,
                             bias=s["nt16p"][:], scale=1.0)
        wt = work.tile([P, WMAX // P, P], BF16, tag="wt", name="wt", bufs=4)
        nc.sync.dma_start(wt[:, 0:w // P, :], wmask[:, 0:w], transpose=True)
        s["wt"] = wt

    def st_matmuls(t):
        s = state[t]
        w = s["w"]
        wt = s["wt"]
        # acc and the window feature-total share one PSUM bank: [:, 0:P]
        # accumulates fbf^T wmask, [:, P] accumulates fbf^T ones
        acc = psum.tile([NF, P + 1], F32, tag="acc", name="acc", bufs=3)
        j0 = int(jlo[t])
        nj = w // P
        for jl in range(nj):
            nc.tensor.matmul(acc[:, 0:P], lhsT=fbf[:, j0 + jl, :],
                             rhs=wt[:, jl, :], start=(jl == 0),
                             stop=(jl == nj - 1))
        for jl in range(nj):
            nc.tensor.matmul(acc[:, P:P + 1], lhsT=fbf[:, j0 + jl, :],
                             rhs=ones_c[:], start=(jl == 0),
                             stop=(jl == nj - 1))
        s["acc"] = acc

    def st_fixup(t):
        s = state.pop(t)
        # sum_sel = (acc + ftot)/2 : ftot PSUM->SBUF (scaled), then
        # Identity with per-partition bias (both on ACT, close to PSUM)
        ftoth = small.tile([NF, 1], F32, tag="ftoth", name="ftoth")
        nc.scalar.activation(ftoth[:], s["acc"][:, P:P + 1], AF.Copy,
                             bias=0.0, scale=0.5)
        covg = work.tile([NF, P], F32, tag="covg", name="covg", bufs=2)
        nc.scalar.activation(covg[:], s["acc"][:, 0:P], AF.Identity,
                             bias=ftoth[:], scale=0.5)
        ctp = psum.tile([P, NF], F32, tag="ctp", name="ctp", bufs=3)
        nc.tensor.matmul(ctp[:], lhsT=covg[:], rhs=ident18[0:NF, 0:NF],
                         is_transpose=True)
        nc.vector.tensor_copy(cov[:, t, :], ctp[:])

    # ---- eigen phase, split into A (through arctan input) and B
    # (arctan onward), emitted as generators so ops spread across steps.
    # Grouping all Sqrt work (A) apart from Arctan/Sin work (B) avoids ACT
    # table-set thrashing; B runs once for all chunks at the end.
    vec = nc.vector

    def tt_(out, a, b, op):
        vec.tensor_tensor(out=out, in0=a, in1=b, op=op)

    def tg_(out, a, b, op):
        # independent (off-critical-chain) eigen products go to GPSIMD
        nc.gpsimd.tensor_tensor(out=out, in0=a, in1=b, op=op)

    def amul(out, a, scale, bias=0.0):
        nc.scalar.activation(out, a, AF.Copy, bias=bias, scale=scale)

    def emit_eigen_A(t0, t1, ec):
        TR = t1 - t0
        covh = cov[:, t0:t1, :]

        def et(name, shape=None):
            return epool.tile(shape or [P, TR], F32, tag=f"{name}_{t0}",
                              name=f"{name}_{t0}")

        ec.update(t0=t0, TR=TR, et=et)
        S = et("S", [P, TR, 9])
        tt_(S[:], covh[:, :, 0:9], covh[:, :, 9:18], OP.add)
        Sq = et("Sq", [P, TR, 3])
        amul(Sq[:], S[:, :, 0:3], 0.25)
        yield
        cm = et("cm", [P, TR, 6])
        tmps = [et(f"cmt{i}") for i in range(6)]
        for i, (a, b) in enumerate(pairs):
            tg_(tmps[i][:], Sq[:, :, a], Sq[:, :, b], OP.mult)
        yield
        for i in range(6):
            tt_(cm[:, :, i], S[:, :, 3 + i], tmps[i][:], OP.subtract)
        yield
        cxx, cyy, czz = cm[:, :, 0], cm[:, :, 1], cm[:, :, 2]
        cxy, cxz, cyz = cm[:, :, 3], cm[:, :, 4], cm[:, :, 5]

        q = et("q")
        q1 = et("q1")
        tt_(q1[:], cxx, cyy, OP.add)
        tt_(q1[:], q1[:], czz, OP.add)
        amul(q[:], q1[:], 1.0 / 3.0)
        b00, b11, b22 = et("b00"), et("b11"), et("b22")
        tt_(b00[:], cxx, q[:], OP.subtract)
        tt_(b11[:], cyy, q[:], OP.subtract)
        tt_(b22[:], czz, q[:], OP.subtract)
        yield
        # p2 = b00^2+b11^2+b22^2 + 2(cxy^2+cxz^2+cyz^2)
        pa, pb, pc_ = et("pa"), et("pb"), et("pc2")
        oa, ob, oc = et("oa"), et("ob"), et("oc")
        tg_(pa[:], b00[:], b00[:], OP.mult)
        tg_(pb[:], b11[:], b11[:], OP.mult)
        tt_(pc_[:], b22[:], b22[:], OP.mult)
        tg_(oa[:], cxy, cxy, OP.mult)
        tg_(ob[:], cxz, cxz, OP.mult)
        tt_(oc[:], cyz, cyz, OP.mult)
        yield
        p2 = et("p2")
        s1, s3 = et("s1"), et("s3")
        tg_(s1[:], pa[:], pb[:], OP.add)
        tt_(s1[:], s1[:], pc_[:], OP.add)
        tg_(s3[:], oa[:], ob[:], OP.add)
        tt_(s3[:], s3[:], oc[:], OP.add)
        s5 = et("s5")
        amul(s5[:], s3[:], 2.0)
        tt_(p2[:], s1[:], s5[:], OP.add)
        p = et("p")
        nc.scalar.activation(p[:], p2[:], AF.Sqrt, bias=0.0, scale=1.0 / 6.0)
        yield
        pc = et("pc")
        vec.tensor_scalar(out=pc[:], in0=p[:], scalar1=1e-30, scalar2=None,
                          op0=OP.max)
        ip = et("ip")
        vec.reciprocal(ip[:], pc[:])
        p2x = et("p2x")
        amul(p2x[:], p[:], 2.0)
        # det(A - qI)
        d1, d3, d4 = et("d1"), et("d3"), et("d4")
        tt_(d1[:], b11[:], b22[:], OP.mult)
        tt_(d3[:], d1[:], oc[:], OP.subtract)
        tt_(d4[:], b00[:], d3[:], OP.mult)
        e1, e2, e3, e4 = et("e1"), et("e2"), et("e3"), et("e4")
        tg_(e1[:], cxy, b22[:], OP.mult)
        tg_(e2[:], cyz, cxz, OP.mult)
        tt_(e3[:], e1[:], e2[:], OP.subtract)
        tt_(e4[:], cxy, e3[:], OP.mult)
        yield
        f1, f2, f3, f4 = et("f1"), et("f2"), et("f3"), et("f4")
        tg_(f1[:], cxy, cyz, OP.mult)
        tg_(f2[:], b11[:], cxz, OP.mult)
        tt_(f3[:], f1[:], f2[:], OP.subtract)
        tt_(f4[:], cxz, f3[:], OP.mult)
        det = et("det")
        tt_(det[:], d4[:], e4[:], OP.subtract)
        tt_(det[:], det[:], f4[:], OP.add)
        yield
        # r = clamp(det * ip^3 / 2, -1, 1)
        i2, i3 = et("i2"), et("i3")
        tt_(i2[:], ip[:], ip[:], OP.mult)
        tt_(i3[:], i2[:], ip[:], OP.mult)
        r = et("r")
        tt_(r[:], det[:], i3[:], OP.mult)
        vec.tensor_scalar(out=r[:], in0=r[:], scalar1=0.5, scalar2=1.0,
                          op0=OP.mult, op1=OP.min)
        vec.tensor_scalar(out=r[:], in0=r[:], scalar1=-1.0, scalar2=None,
                          op0=OP.max)
        yield
        # acos(r) via octant-reduced arctan: A ends at the arctan INPUT
        rr = et("rr")
        tt_(rr[:], r[:], r[:], OP.mult)
        aab = et("aab")
        nc.scalar.activation(aab[:], rr[:], AF.Sqrt, bias=0.0, scale=1.0)
        rr2 = et("rr2")
        vec.tensor_scalar(out=rr2[:], in0=rr[:], scalar1=-1.0, scalar2=1.0,
                          op0=OP.mult, op1=OP.add)
        s = et("s")
        nc.scalar.activation(s[:], rr2[:], AF.Sqrt, bias=0.0, scale=1.0)
        mn, mx = et("mn"), et("mx")
        tt_(mn[:], aab[:], s[:], OP.min)
        tt_(mx[:], aab[:], s[:], OP.max)
        imx = et("imx")
        vec.reciprocal(imx[:], mx[:])
        ratio = et("ratio")
        tt_(ratio[:], mn[:], imx[:], OP.mult)
        ec.update(q=q, p2x=p2x, r=r, s=s, aab=aab, ratio=ratio)

    def emit_eigen_B(ecs):
        # all arctans first, then the (table-free) fold chains, then all
        # sins, then eigenvalue assembly + output DMA per chunk
        for ec in ecs:
            th = ec["et"]("th")
            nc.scalar.activation(th[:], ec["ratio"][:], AF.Arctan, bias=0.0,
                                 scale=1.0)
            ec["th"] = th
        for ec in ecs:
            et = ec["et"]
            mk = et("mk")
            tt_(mk[:], ec["s"][:], ec["aab"][:], OP.is_gt)
            u1 = et("u1")
            amul(u1[:], ec["th"][:], -2.0, PI / 2)
            u2 = et("u2")
            tt_(u2[:], mk[:], u1[:], OP.mult)
            th2 = et("th2")
            tt_(th2[:], ec["th"][:], u2[:], OP.add)
            mk2 = et("mk2")
            vec.tensor_scalar(out=mk2[:], in0=ec["r"][:], scalar1=0.0,
                              scalar2=None, op0=OP.is_lt)
            u3 = et("u3")
            amul(u3[:], th2[:], -2.0, PI)
            u4 = et("u4")
            tt_(u4[:], mk2[:], u3[:], OP.mult)
            th3 = et("th3")
            tt_(th3[:], th2[:], u4[:], OP.add)
            phi = et("phi")
            amul(phi[:], th3[:], 1.0 / 3.0)
            ec["phi"] = phi
        for ec in ecs:
            et = ec["et"]
            c1, c3 = et("c1"), et("c3")
            nc.scalar.activation(c1[:], ec["phi"][:], AF.Sin,
                                 bias=bias_c[:, 0:1], scale=1.0)
            nc.scalar.activation(c3[:], ec["phi"][:], AF.Sin,
                                 bias=bias_c[:, 1:2], scale=1.0)
            ec["c1"], ec["c3"] = c1, c3
        out_r = out_ap.rearrange("(p t) k -> p t k", p=P)
        for ec in ecs:
            et = ec["et"]
            t0, TR, q, p2x = ec["t0"], ec["TR"], ec["q"], ec["p2x"]
            eigs = et("eigs", [P, TR, 3])
            g1, g2 = et("g1"), et("g2")
            tt_(g1[:], p2x[:], ec["c1"][:], OP.mult)
            tt_(eigs[:, :, 0], g1[:], q[:], OP.add)
            tt_(g2[:], p2x[:], ec["c3"][:], OP.mult)
            tt_(eigs[:, :, 2], q[:], g2[:], OP.subtract)
            q3 = et("q3")
            amul(q3[:], q[:], 3.0)
            tt_(q3[:], q3[:], eigs[:, :, 0], OP.subtract)
            tt_(eigs[:, :, 1], q3[:], eigs[:, :, 2], OP.subtract)
            nsp = min(4, TR)
            for d in range(nsp):
                sl = slice(t0 + d * TR // nsp, t0 + (d + 1) * TR // nsp)
                sle = slice(d * TR // nsp, (d + 1) * TR // nsp)
                nc.scalar.dma_start(out_r[:, sl, :], eigs[:, sle, :])

    # ---- main loop: skewed stage emission + spread eigen ----
    # emit oldest-tile stages first within each step (drain order) so no
    # engine queue holds younger work ahead of older dependencies
    stages = [(6, st_fixup), (5, st_matmuls), (4, st_mask), (3, st_sel),
              (1, st_neg), (0, st_squares)]
    chunks = [(0, T // 2), (T // 2, T - 6), (T - 6, T)]
    ecs = [{} for _ in chunks]
    gens = []
    for step in range(T + 6):
        for skew, fn in stages:
            tau = step - skew
            if 0 <= tau < T:
                fn(tau)
        for ci, (c0, c1_) in enumerate(chunks):
            if step == c1_ + 6:      # one step after fixup(c1-1)
                gens.append(emit_eigen_A(c0, c1_, ecs[ci]))
        for g in list(gens):
            try:
                next(g)
            except StopIteration:
                gens.remove(g)
    for g in gens:
        for _ in g:
            pass
    g3 = emit_eigen_A(*chunks[-1], ecs[-1]) if not ecs[-1] else None
    if g3 is not None:
        for _ in g3:
            pass
    emit_eigen_B(ecs)


def _emit_with_ident(ctx, tc, out_ap, uvt_ap, uvq_ap, xq_ap, M, jlo, jhi,
                     c0e, c1e):
    # identity matrix for the PE cov-transpose, built once
    nc = tc.nc
    const = ctx.enter_context(tc.tile_pool(name="identc", bufs=1))
    iota_a = const.tile([P, P], I32, tag="iota_a", name="iota_a")
    nc.gpsimd.iota(iota_a[:], pattern=[[1, P]], base=0, channel_multiplier=0)
    iota_b = const.tile([P, 1], I32, tag="iota_b", name="iota_b")
    nc.gpsimd.iota(iota_b[:], pattern=[[1, 1]], base=0, channel_multiplier=1)
    iota_af = const.tile([P, P], F32, tag="iota_af", name="iota_af")
    nc.gpsimd.tensor_copy(iota_af[:], iota_a[:])
    iota_bf = const.tile([P, 1], F32, tag="iota_bf", name="iota_bf")
    nc.gpsimd.tensor_copy(iota_bf[:], iota_b[:])
    ident = const.tile([P, P], F32, tag="ident", name="ident")
    nc.gpsimd.tensor_scalar(out=ident[:], in0=iota_af[:],
                            scalar1=iota_bf[:, 0:1],
                            scalar2=None, op0=OP.is_equal)
    _emit(ctx, tc, out_ap, uvt_ap, uvq_ap, xq_ap, M, ident, jlo, jhi,
          c0e, c1e)


def build_nc(M, jlo, jhi, c0e, c1e):
    nc = bacc.Bacc("TRN2", target_bir_lowering=False, debug=False,
                   enable_asserts=False)
    uvt_ap = nc.dram_tensor("uvt", (2, M), F32, kind="ExternalInput").ap()
    uvq_ap = nc.dram_tensor("uvq", (M, 2), F32, kind="ExternalInput").ap()
    xq_ap = nc.dram_tensor("xq", (M, 3), F32, kind="ExternalInput").ap()
    out_ap = nc.dram_tensor("out", (M, 3), F32, kind="ExternalOutput").ap()
    with tile.TileContext(nc) as tc:
        with ExitStack() as ctx:
            _emit_with_ident(ctx, tc, out_ap, uvt_ap, uvq_ap, xq_ap, M,
                             jlo, jhi, c0e, c1e)
    nc.compile()
    return nc


_NC_CACHE = {}


def _get_nc(M, jlo, jhi, c0e, c1e):
    key = (M, tuple(jlo), tuple(jhi), tuple(c0e), tuple(c1e))
    if key not in _NC_CACHE:
        _NC_CACHE[key] = build_nc(M, jlo, jhi, c0e, c1e)
    return _NC_CACHE[key]


def run(X, uv, trace: bool = False):
    B, M, _ = X.shape
    X = np.ascontiguousarray(X, dtype=np.float32)
    uv = np.ascontiguousarray(uv, dtype=np.float32)
    perms = []
    jlo = jhi = c0e = c1e = None
    for b in range(B):
        perm, jl, jh, ce0, ce1 = _batch_windows(uv[b])
        perms.append(perm)
        jlo = jl if jlo is None else np.minimum(jlo, jl)
        jhi = jh if jhi is None else np.maximum(jhi, jh)
        c0e = ce0 if c0e is None else np.minimum(c0e, ce0)
        c1e = ce1 if c1e is None else np.maximum(c1e, ce1)
    nc = _get_nc(M, jlo, jhi, c0e, c1e)
    T = M // P
    in_maps = []
    for b in range(B):
        us = uv[b][perms[b]]
        xs = X[b][perms[b]]
        in_maps.append({
            "uvt": np.ascontiguousarray(us.T),
            "uvq": np.ascontiguousarray(
                us.reshape(T, P, 2).transpose(1, 0, 2).reshape(M, 2)),
            "xq": np.ascontiguousarray(
                xs.reshape(T, P, 3).transpose(1, 0, 2).reshape(M, 3)),
        })
    res = run_bass_kernel_spmd(nc, in_maps, core_ids=list(range(B)),
                               trace=trace)
    out = np.empty((B, M, 3), np.float32)
    for b in range(B):
        o = res.results[b]["out"]
        out[b][perms[b]] = (o.reshape(P, T, 3).transpose(1, 0, 2)
                            .reshape(M, 3))
    return out, res


def kernel(X, uv):
    X = np.asarray(X)
    uv = np.asarray(uv)
    out, _ = run(X, uv, trace=False)
    return out.astype(np.float32)


# revision 43
# speedup vs baseline: 1.2099x; 1.2099x over previous
"""Trainium2 Bass kernel for DiffGeomPropsApprox (within-batch uv-space 16-NN
-> neighborhood covariance of X -> descending symmetric-3x3 eigenvalues).

Sharding: data-parallel over batch B=8, one batch per NeuronCore (8 cores).

Spatial windowing: host-side, each batch's points are sorted into a
serpentine strip order (strips = v-rank quantiles, u alternating
direction). A conservative per-query radius bound R(q) - refined to the
16th-smallest distance within a provably-sufficient cell box - yields,
per 128-query tile, a contiguous sorted-index window that contains every
query's exact 16-NN. The device only computes distances/top-k/mask/matmul
over that window (~5-7 of 32 tiles). Windows are computed at runtime from
the actual inputs (union across batches; one SPMD program).

Per-query-tile device pipeline (software-pipelined stage emission with
skews squares@0 / neg@+1 / sel@+3 / mask@+4 / matmul@+5 / fixup@+6, each
engine's in-order queue only sees work whose producers ran earlier):
  ACT:   squ=(u_c-u_q)^2, sqv=(v_c-v_q)^2 (Square, per-partition bias;
         exact reference f32 rounding)
  neg:   negdm = -(squ+sqv) exactly; production rotates GPSIMD-double /
         DVE fused custom op (LN_BWD_DX_ANT) / ACT-neg+GPSIMD-sub
  DVE:   max8 -> match_replace -> max8 over negdm -> -d16 (16th smallest)
  ACT:   mask = Sign(negdm + d16*(1+2^-22)) in {-1,+1} bf16
  DMA:   x-bar transpose mask -> [c-part, q]
  PE:    per window tile j: matmul acc += fbf_j^T wmask_j (18 features =
         9 x bf16 hi/lo) and ftot += fbf_j^T ones (separate PSUM groups)
  ACT:   fixup sum_sel = (acc+ftot)/2 via Identity bias-add; PE transpose;
         DVE slab copy.
Startup: partition-broadcast of u/v rows via PE fp32 ones-outer-product
(exact), PSUM-chunked; all input DMAs contiguous (host pre-transposes).
Eigenvalues: closed-form trig method in 3 chunks; sqrt phase (A) overlaps
the main loop via generator-spread emission, arctan/sin phase (B) runs
once at the end grouped by ACT table-set to avoid LUT reload thrash.
"""

from contextlib import ExitStack

import numpy as np

import concourse.bass as bass
import concourse.tile as tile
from concourse import bacc, mybir
from concourse.alu_op_type import AluOpType
from concourse.bass_utils import run_bass_kernel_spmd
from concourse.dve_ops import LN_BWD_DX_ANT

F32 = mybir.dt.float32
BF16 = mybir.dt.bfloat16
I32 = mybir.dt.int32
AF = mybir.ActivationFunctionType
OP = AluOpType

P = 128
K = 16
NEG_BIG = -3.0e38
PI = float(np.pi)
EPS_REL = 1.0 + 2.0 ** -22

G_STRIPS = 32          # equal-count strips (by v-rank)
H_INV = 256            # grid resolution for the coarse R(q) bound
H2_INV = 32            # cell-list resolution for the refined bound


# --------------------------- host-side windowing --------------------------- #

def _strip_perm(uv, G=G_STRIPS):
    """Serpentine strip order: strip = v-rank quantile, u asc/desc."""
    M = uv.shape[0]
    u, v = uv[:, 0], uv[:, 1]
    rank_v = np.empty(M, np.int64)
    rank_v[np.argsort(v, kind="stable")] = np.arange(M)
    strip = rank_v * G // M
    ukey = np.where(strip % 2 == 0, u, -u)
    return np.lexsort((ukey, strip))


def _geom_R(uv, h_inv=H_INV):
    """Coarse conservative bound on the 16-NN radius: smallest (2r+1)^2
    cell box centered on the point's cell holding >= K points; any K
    points in that box lie within (r+1)*h*sqrt(2)."""
    M = uv.shape[0]
    u, v = uv[:, 0], uv[:, 1]
    ci = np.minimum((u * h_inv).astype(np.int64), h_inv - 1)
    cj = np.minimum((v * h_inv).astype(np.int64), h_inv - 1)
    H = np.zeros((h_inv, h_inv), np.int64)
    np.add.at(H, (ci, cj), 1)
    S = np.zeros((h_inv + 1, h_inv + 1), np.int64)
    S[1:, 1:] = H.cumsum(0).cumsum(1)
    R = np.full(M, np.sqrt(2.0))
    done = np.zeros(M, bool)
    for rho in range(1, h_inv):
        i0 = np.clip(ci - rho, 0, h_inv); i1 = np.clip(ci + rho + 1, 0, h_inv)
        j0 = np.clip(cj - rho, 0, h_inv); j1 = np.clip(cj + rho + 1, 0, h_inv)
        cnt = S[i1, j1] - S[i0, j1] - S[i1, j0] + S[i0, j0]
        new = (~done) & (cnt >= K)
        R[new] = (rho + 1) / h_inv * np.sqrt(2.0)
        done |= new
        if done.all():
            break
    return R


def _refine_R(uv, R0, h_inv=H2_INV):
    """Tighten R to the 16th-smallest distance within the cell box that
    provably covers disc(q, R0).  disc(q, R0) holds >= K points (by R0's
    construction), the box covers the disc, so the 16th-nearest within
    the box is <= R0 and >= the true 16-NN radius."""
    M = uv.shape[0]
    u, v = uv[:, 0].astype(np.float64), uv[:, 1].astype(np.float64)
    ci = np.minimum((u * h_inv).astype(np.int64), h_inv - 1)
    cj = np.minimum((v * h_inv).astype(np.int64), h_inv - 1)
    cell = ci * h_inv + cj
    order = np.argsort(cell, kind="stable")
    csort = cell[order]
    ncell = h_inv * h_inv
    starts = np.searchsorted(csort, np.arange(ncell + 1))
    cnts = np.diff(starts)
    cmax = int(cnts.max())
    C = np.full((ncell, cmax), -1, np.int64)
    for c in range(ncell):
        C[c, :cnts[c]] = order[starts[c]:starts[c + 1]]
    rho = np.ceil(R0 * h_inv).astype(np.int64)
    R1 = np.empty(M, np.float64)
    for rv in np.unique(rho):
        sel = np.where(rho == rv)[0]
        offs = [(di, dj) for di in range(-rv, rv + 1)
                for dj in range(-rv, rv + 1)]
        cand = np.empty((len(sel), len(offs) * cmax), np.int64)
        for k, (di, dj) in enumerate(offs):
            ii = np.clip(ci[sel] + di, 0, h_inv - 1)
            jj = np.clip(cj[sel] + dj, 0, h_inv - 1)
            # out-of-range clamps may duplicate cells; harmless (extra
            # candidates only shrink the 16th distance toward truth; they
            # are real points so the bound stays valid)
            cand[:, k * cmax:(k + 1) * cmax] = C[ii * h_inv + jj]
        pad = cand < 0
        cid = np.where(pad, 0, cand)
        d2 = (u[sel, None] - u[cid]) ** 2 + (v[sel, None] - v[cid]) ** 2
        d2[pad] = np.inf
        # dedupe isn't needed for validity, but clamp-duplicated points
        # could make the 16th-smallest too small -> not conservative.
        # Sort candidate ids per row and inf-out repeats.
        si = np.argsort(cand, axis=1, kind="stable")
        cs = np.take_along_axis(cand, si, 1)
        dup = np.zeros_like(pad)
        dup[:, 1:] = cs[:, 1:] == cs[:, :-1]
        ds = np.take_along_axis(d2, si, 1)
        ds[dup | (cs < 0)] = np.inf
        R1[sel] = np.sqrt(np.partition(ds, K - 1, axis=1)[:, K - 1])
    return np.minimum(R0, np.nextafter(R1.astype(np.float32),
                                       np.float32(np.inf)))


def _batch_windows(uv, G=G_STRIPS):
    """perm + per-tile [jlo, jhi] (tile units) windows containing, for
    every query in the tile, all points within R(q) (hence its 16-NN)."""
    M = uv.shape[0]
    T = M // P
    spts = M // G
    perm = _strip_perm(uv, G)
    us, vs = uv[perm, 0], uv[perm, 1]
    R = _refine_R(uv, _geom_R(uv))
    Rq = R[perm]
    vmin = vs.reshape(G, spts).min(1)
    vmax = vs.reshape(G, spts).max(1)
    slo = np.clip(np.searchsorted(vmax, vs - Rq, side="left"), 0, G - 1)
    shi = np.clip(np.searchsorted(vmin, vs + Rq, side="right") - 1, 0, G - 1)
    plo = np.empty(M, np.int64)
    phi = np.empty(M, np.int64)
    for s in range(G):
        base = s * spts
        su = us[base:base + spts]
        asc = (s % 2 == 0)
        sua = su if asc else su[::-1]
        for sel, is_lo in ((slo == s, True), (shi == s, False)):
            if not sel.any():
                continue
            ulo = us[sel] - Rq[sel]
            uhi = us[sel] + Rq[sel]
            if is_lo:
                off = (np.searchsorted(sua, ulo, side="left") if asc else
                       spts - np.searchsorted(sua, uhi, side="right"))
                plo[sel] = base + np.clip(off, 0, spts - 1)
            else:
                off = (np.searchsorted(sua, uhi, side="right") - 1 if asc else
                       spts - 1 - np.searchsorted(sua, ulo, side="left"))
                phi[sel] = base + np.clip(off, 0, spts - 1)
    phi = np.maximum(phi, plo)
    jlo = np.empty(T, np.int64)
    jhi = np.empty(T, np.int64)
    c0e = np.empty(T, np.int64)
    c1e = np.empty(T, np.int64)
    for t in range(T):
        c0e[t] = plo[t * P:(t + 1) * P].min()
        c1e[t] = phi[t * P:(t + 1) * P].max() + 1
        jlo[t] = c0e[t] // P
        jhi[t] = (c1e[t] - 1) // P
    return perm, jlo, jhi, c0e, c1e


# ----------------------------- device kernel ------------------------------- #

def _emit(ctx, tc, out_ap, uvt_ap, uvq_ap, xq_ap, M, ident18, jlo, jhi,
          c0e, c1e):
    nc = tc.nc
    T = M // P
    NF = 18
    WTS = [int(jhi[t] - jlo[t] + 1) for t in range(T)]
    WMAX = max(WTS) * P
    # column-exact sub-ranges within each tile window (8-aligned) for the
    # pointwise passes; the pad columns only exist in the mask (set to -1)
    A0 = [(int(c0e[t]) - int(jlo[t]) * P) // 8 * 8 for t in range(T)]
    A1 = [min(-(-(int(c1e[t]) - int(jlo[t]) * P) // 8) * 8, WTS[t] * P)
          for t in range(T)]

    const = ctx.enter_context(tc.tile_pool(name="const", bufs=1))
    work = ctx.enter_context(tc.tile_pool(name="work", bufs=2))
    small = ctx.enter_context(tc.tile_pool(name="small", bufs=14))
    psum = ctx.enter_context(tc.tile_pool(name="psum", bufs=2, space="PSUM"))
    epool = ctx.enter_context(tc.tile_pool(name="eig", bufs=1))

    # ---- broadcast candidate coords across partitions ----
    # uvt is host-transposed [2, M]: step-0 loads are contiguous rows.
    # Doubling chain, column-split across two queues per coordinate.
    # slab layouts [P, T, k]: host pre-transposed so loads are contiguous
    uv_slab = const.tile([P, T, 2], F32, tag="uv_slab")
    uv_r = uvq_ap.rearrange("(p t) k -> p t k", p=P)
    x_slab = const.tile([P, T, 3], F32, tag="x_slab")
    x_r = xq_ap.rearrange("(p t) k -> p t k", p=P)
    nc.sync.dma_start(uv_slab[:], uv_r[:])
    nc.scalar.dma_start(x_slab[:], x_r[:])

    # partition-broadcast of u/v rows via PE fp32 outer product with a ones
    # column (exact: 1.0*x). 512-col PSUM chunks, copied out on ACT/DVE.
    u_b = const.tile([P, M], F32, tag="u_b")
    v_b = const.tile([P, M], F32, tag="v_b")
    ones1 = const.tile([1, P], F32, tag="ones1")
    nc.gpsimd.memset(ones1[:], 1.0)
    urow = const.tile([1, M], F32, tag="urow")
    vrow = const.tile([1, M], F32, tag="vrow")
    nc.sync.dma_start(urow[:], uvt_ap[0:1, :])
    nc.scalar.dma_start(vrow[:], uvt_ap[1:2, :])
    BC = 512
    for ci, c in enumerate(range(0, M, BC)):
        for row, dst in ((urow, u_b), (vrow, v_b)):
            pb = psum.tile([P, BC], F32, tag="bc", name="bc", bufs=2)
            nc.tensor.matmul(pb[:], lhsT=ones1[:], rhs=row[0:1, c:c + BC],
                             start=True, stop=True)
            if (ci + (0 if dst is u_b else 1)) % 2 == 0:
                nc.vector.tensor_copy(dst[:, c:c + BC], pb[:])
            else:
                nc.scalar.copy(dst[:, c:c + BC], pb[:])
    nuv = const.tile([P, T, 2], F32, tag="nuv")
    nc.vector.tensor_scalar(out=nuv[:], in0=uv_slab[:], scalar1=-1.0,
                            scalar2=None, op0=OP.mult)

    # ---- features: [x y z x2 y2 z2 xy xz yz] as bf16 hi/lo ----
    pairs = [(0, 0), (1, 1), (2, 2), (0, 1), (0, 2), (1, 2)]
    fsl = work.tile([P, T, 9], F32, tag="fsl", name="fsl", bufs=1)
    nc.vector.tensor_copy(fsl[:, :, 0:3], x_slab[:])
    for i, (a, b) in enumerate(pairs):
        nc.vector.tensor_tensor(out=fsl[:, :, 3 + i], in0=x_slab[:, :, a],
                                in1=x_slab[:, :, b], op=OP.mult)
    fbf = const.tile([P, T, NF], BF16, tag="fbf")
    nc.vector.tensor_copy(fbf[:, :, 0:9], fsl[:])
    fhi32 = work.tile([P, T, 9], F32, tag="fhi32", name="fhi32", bufs=1)
    nc.vector.tensor_copy(fhi32[:], fbf[:, :, 0:9])
    nc.vector.tensor_tensor(out=fbf[:, :, 9:18], in0=fsl[:], in1=fhi32[:],
                            op=OP.subtract)

    ones_c = const.tile([P, 1], BF16, tag="ones_c")
    nc.gpsimd.memset(ones_c[:], 1.0)
    zeros = const.tile([P, WMAX], F32, tag="zeros")
    nc.gpsimd.memset(zeros[:], 0.0)
    bias_c = const.tile([P, 2], F32, tag="bias_c")
    nc.gpsimd.memset(bias_c[:, 0:1], PI / 2)
    nc.gpsimd.memset(bias_c[:, 1:2], PI / 6)

    cov = const.tile([P, T, NF], F32, tag="cov")

    # ---- pipeline stages -------------------------------------------------
    state = {}

    def st_squares(t):
        c0 = int(jlo[t]) * P
        a0, a1 = A0[t], A1[t]
        squ = work.tile([P, WMAX], F32, tag="sq", name="squ", bufs=8)
        nc.scalar.activation(squ[:, a0:a1], u_b[:, c0 + a0:c0 + a1],
                             AF.Square, bias=nuv[:, t, 0:1], scale=1.0)
        sqv = work.tile([P, WMAX], F32, tag="sq", name="sqv", bufs=8)
        nc.scalar.activation(sqv[:, a0:a1], v_b[:, c0 + a0:c0 + a1],
                             AF.Square, bias=nuv[:, t, 1:2], scale=1.0)
        state[t] = {"squ": squ, "sqv": sqv, "w": WTS[t] * P}

    def st_neg(t):
        # negdm = -(squ + sqv), exact; production rotates across engines:
        #   t%3==0: GPSIMD (0-squ) then (nsq-sqv)
        #   t%3==1: DVE fused custom op (squ - sqv*-1 - 0) * -1
        #   t%3==2: ACT -squ, then GPSIMD (nsq - sqv)
        s = state[t]
        a0, a1 = A0[t], A1[t]
        negdm = work.tile([P, WMAX], F32, tag="negdm", name="negdm", bufs=6)
        r = t % 3
        if r == 1:
            nc.vector._custom_dve(LN_BWD_DX_ANT, out=negdm[:, a0:a1],
                                  in0=s["squ"][:, a0:a1],
                                  in1=s["sqv"][:, a0:a1],
                                  s0=-1.0, s1=0.0, imm2=-1.0)
        else:
            nsq = work.tile([P, WMAX], F32, tag="dm", name="nsq", bufs=4)
            if r == 0:
                nc.gpsimd.tensor_tensor(out=nsq[:, a0:a1],
                                        in0=zeros[:, a0:a1],
                                        in1=s["squ"][:, a0:a1],
                                        op=OP.subtract)
            else:
                nc.scalar.activation(nsq[:, a0:a1], s["squ"][:, a0:a1],
                                     AF.Copy, bias=0.0, scale=-1.0)
            nc.gpsimd.tensor_tensor(out=negdm[:, a0:a1], in0=nsq[:, a0:a1],
                                    in1=s["sqv"][:, a0:a1], op=OP.subtract)
        s["negdm"] = negdm

    def st_sel(t):
        s = state[t]
        a0, a1 = A0[t], A1[t]
        negdm = s["negdm"]
        m1 = small.tile([P, 8], F32, tag="m1", name="m1")
        nc.vector.max(m1[:], negdm[:, a0:a1])
        mr = work.tile([P, WMAX], F32, tag="mr", name="mr", bufs=3)
        nc.vector.match_replace(mr[:, a0:a1], m1[:], negdm[:, a0:a1],
                                NEG_BIG)
        m2 = small.tile([P, 8], F32, tag="m2", name="m2")
        nc.vector.max(m2[:], mr[:, a0:a1])
        # Sign-mask bias: +d16*(1+2^-22)  (m2[7] = -d16)
        nt16p = small.tile([P, 1], F32, tag="nt16p", name="nt16p")
        nc.vector.tensor_scalar(out=nt16p[:], in0=m2[:, 7:8],
                                scalar1=-EPS_REL, scalar2=None, op0=OP.mult)
        s["nt16p"] = nt16p

    def st_mask(t):
        s = state[t]
        w = s["w"]
        a0, a1 = A0[t], A1[t]
        # {-1,+1} mask in bf16 on ACT (Sign LUT); +1 iff d <= d16*(1+eps).
        # Pad columns outside [a0,a1) hold -1 (unselected) so the ftot
        # correction stays consistent over the full tile window.
        wmask = work.tile([P, WMAX], BF16, tag="wmask", name="wmask", bufs=3)
        if a0 > 0:
            nc.vector.memset(wmask[:, 0:a0], -1.0)
        if a1 < w:
            nc.vector.memset(wmask[:, a1:w], -1.0)
        nc.scalar.activation(wmask[:, a0:a1], s["negdm"][:, a0:a1], AF.Sign,
                             bias=s["nt16p"][:], scale=1.0)
        wt = work.tile([P, WMAX // P, P], BF16, tag="wt", name="wt", bufs=4)
        nc.sync.dma_start(wt[:, 0:w // P, :], wmask[:, 0:w], transpose=True)
        s["wt"] = wt

    def st_matmuls(t):
        s = state[t]
        w = s["w"]
        wt = s["wt"]
        # acc and the window feature-total share one PSUM bank: [:, 0:P]
        # accumulates fbf^T wmask, [:, P] accumulates fbf^T ones
        acc = psum.tile([NF, P + 1], F32, tag="acc", name="acc", bufs=3)
        j0 = int(jlo[t])
        nj = w // P
        for jl in range(nj):
            nc.tensor.matmul(acc[:, 0:P], lhsT=fbf[:, j0 + jl, :],
                             rhs=wt[:, jl, :], start=(jl == 0),
                             stop=(jl == nj - 1))
        for jl in range(nj):
            nc.tensor.matmul(acc[:, P:P + 1], lhsT=fbf[:, j0 + jl, :],
                             rhs=ones_c[:], start=(jl == 0),
                             stop=(jl == nj - 1))
        s["acc"] = acc

    def st_fixup(t):
        s = state.pop(t)
        # sum_sel = (acc + ftot)/2 : ftot PSUM->SBUF (scaled), then
        # Identity with per-partition bias (both on ACT, close to PSUM)
        ftoth = small.tile([NF, 1], F32, tag="ftoth", name="ftoth")
        nc.scalar.activation(ftoth[:], s["acc"][:, P:P + 1], AF.Copy,
                             bias=0.0, scale=0.5)
        covg = work.tile([NF, P], F32, tag="covg", name="covg", bufs=2)
        nc.scalar.activation(covg[:], s["acc"][:, 0:P], AF.Identity,
                             bias=ftoth[:], scale=0.5)
        ctp = psum.tile([P, NF], F32, tag="ctp", name="ctp", bufs=3)
        nc.tensor.matmul(ctp[:], lhsT=covg[:], rhs=ident18[0:NF, 0:NF],
                         is_transpose=True)
        nc.vector.tensor_copy(cov[:, t, :], ctp[:])

    # ---- eigen phase, split into A (through arctan input) and B
    # (arctan onward), emitted as generators so ops spread across steps.
    # Grouping all Sqrt work (A) apart from Arctan/Sin work (B) avoids ACT
    # table-set thrashing; B runs once for all chunks at the end.
    vec = nc.vector

    def tt_(out, a, b, op):
        vec.tensor_tensor(out=out, in0=a, in1=b, op=op)

    def tg_(out, a, b, op):
        # independent (off-critical-chain) eigen products go to GPSIMD
        nc.gpsimd.tensor_tensor(out=out, in0=a, in1=b, op=op)

    def amul(out, a, scale, bias=0.0):
        nc.scalar.activation(out, a, AF.Copy, bias=bias, scale=scale)

    def emit_eigen_A(t0, t1, ec):
        TR = t1 - t0
        covh = cov[:, t0:t1, :]

        def et(name, shape=None):
            return epool.tile(shape or [P, TR], F32, tag=f"{name}_{t0}",
                              name=f"{name}_{t0}")

        ec.update(t0=t0, TR=TR, et=et)
        S = et("S", [P, TR, 9])
        tt_(S[:], covh[:, :, 0:9], covh[:, :, 9:18], OP.add)
        Sq = et("Sq", [P, TR, 3])
        amul(Sq[:], S[:, :, 0:3], 0.25)
        yield
        cm = et("cm", [P, TR, 6])
        tmps = [et(f"cmt{i}") for i in range(6)]
        for i, (a, b) in enumerate(pairs):
            tg_(tmps[i][:], Sq[:, :, a], Sq[:, :, b], OP.mult)
        yield
        for i in range(6):
            tt_(cm[:, :, i], S[:, :, 3 + i], tmps[i][:], OP.subtract)
        yield
        cxx, cyy, czz = cm[:, :, 0], cm[:, :, 1], cm[:, :, 2]
        cxy, cxz, cyz = cm[:, :, 3], cm[:, :, 4], cm[:, :, 5]

        q = et("q")
        q1 = et("q1")
        tt_(q1[:], cxx, cyy, OP.add)
        tt_(q1[:], q1[:], czz, OP.add)
        amul(q[:], q1[:], 1.0 / 3.0)
        b00, b11, b22 = et("b00"), et("b11"), et("b22")
        tt_(b00[:], cxx, q[:], OP.subtract)
        tt_(b11[:], cyy, q[:], OP.subtract)
        tt_(b22[:], czz, q[:], OP.subtract)
        yield
        # p2 = b00^2+b11^2+b22^2 + 2(cxy^2+cxz^2+cyz^2)
        pa, pb, pc_ = et("pa"), et("pb"), et("pc2")
        oa, ob, oc = et("oa"), et("ob"), et("oc")
        tg_(pa[:], b00[:], b00[:], OP.mult)
        tg_(pb[:], b11[:], b11[:], OP.mult)
        tt_(pc_[:], b22[:], b22[:], OP.mult)
        tg_(oa[:], cxy, cxy, OP.mult)
        tg_(ob[:], cxz, cxz, OP.mult)
        tt_(oc[:], cyz, cyz, OP.mult)
        yield
        p2 = et("p2")
        s1, s3 = et("s1"), et("s3")
        tt_(s1[:], pa[:], pb[:], OP.add)
        tt_(s1[:], s1[:], pc_[:], OP.add)
        tt_(s3[:], oa[:], ob[:], OP.add)
        tt_(s3[:], s3[:], oc[:], OP.add)
        s5 = et("s5")
        amul(s5[:], s3[:], 2.0)
        tt_(p2[:], s1[:], s5[:], OP.add)
        p = et("p")
        nc.scalar.activation(p[:], p2[:], AF.Sqrt, bias=0.0, scale=1.0 / 6.0)
        yield
        pc = et("pc")
        vec.tensor_scalar(out=pc[:], in0=p[:], scalar1=1e-30, scalar2=None,
                          op0=OP.max)
        ip = et("ip")
        vec.reciprocal(ip[:], pc[:])
        p2x = et("p2x")
        amul(p2x[:], p[:], 2.0)
        # det(A - qI)
        d1, d3, d4 = et("d1"), et("d3"), et("d4")
        tt_(d1[:], b11[:], b22[:], OP.mult)
        tt_(d3[:], d1[:], oc[:], OP.subtract)
        tt_(d4[:], b00[:], d3[:], OP.mult)
        e1, e2, e3, e4 = et("e1"), et("e2"), et("e3"), et("e4")
        tg_(e1[:], cxy, b22[:], OP.mult)
        tg_(e2[:], cyz, cxz, OP.mult)
        tt_(e3[:], e1[:], e2[:], OP.subtract)
        tt_(e4[:], cxy, e3[:], OP.mult)
        yield
        f1, f2, f3, f4 = et("f1"), et("f2"), et("f3"), et("f4")
        tg_(f1[:], cxy, cyz, OP.mult)
        tg_(f2[:], b11[:], cxz, OP.mult)
        tt_(f3[:], f1[:], f2[:], OP.subtract)
        tt_(f4[:], cxz, f3[:], OP.mult)
        det = et("det")
        tt_(det[:], d4[:], e4[:], OP.subtract)
        tt_(det[:], det[:], f4[:], OP.add)
        yield
        # r = clamp(det * ip^3 / 2, -1, 1)
        i2, i3 = et("i2"), et("i3")
        tt_(i2[:], ip[:], ip[:], OP.mult)
        tt_(i3[:], i2[:], ip[:], OP.mult)
        r = et("r")
        tt_(r[:], det[:], i3[:], OP.mult)
        vec.tensor_scalar(out=r[:], in0=r[:], scalar1=0.5, scalar2=1.0,
                          op0=OP.mult, op1=OP.min)
        vec.tensor_scalar(out=r[:], in0=r[:], scalar1=-1.0, scalar2=None,
                          op0=OP.max)
        yield
        # acos(r) via octant-reduced arctan: A ends at the arctan INPUT
        rr = et("rr")
        tt_(rr[:], r[:], r[:], OP.mult)
        aab = et("aab")
        nc.scalar.activation(aab[:], rr[:], AF.Sqrt, bias=0.0, scale=1.0)
        rr2 = et("rr2")
        vec.tensor_scalar(out=rr2[:], in0=rr[:], scalar1=-1.0, scalar2=1.0,
                          op0=OP.mult, op1=OP.add)
        s = et("s")
        nc.scalar.activation(s[:], rr2[:], AF.Sqrt, bias=0.0, scale=1.0)
        mn, mx = et("mn"), et("mx")
        tt_(mn[:], aab[:], s[:], OP.min)
        tt_(mx[:], aab[:], s[:], OP.max)
        imx = et("imx")
        vec.reciprocal(imx[:], mx[:])
        ratio = et("ratio")
        tt_(ratio[:], mn[:], imx[:], OP.mult)
        ec.update(q=q, p2x=p2x, r=r, s=s, aab=aab, ratio=ratio)

    def emit_eigen_B(ecs):
        # all arctans first, then the (table-free) fold chains, then all
        # sins, then eigenvalue assembly + output DMA per chunk
        for ec in ecs:
            th = ec["et"]("th")
            nc.scalar.activation(th[:], ec["ratio"][:], AF.Arctan, bias=0.0,
                                 scale=1.0)
            ec["th"] = th
        for ec in ecs:
            et = ec["et"]
            mk = et("mk")
            tt_(mk[:], ec["s"][:], ec["aab"][:], OP.is_gt)
            u1 = et("u1")
            amul(u1[:], ec["th"][:], -2.0, PI / 2)
            u2 = et("u2")
            tt_(u2[:], mk[:], u1[:], OP.mult)
            th2 = et("th2")
            tt_(th2[:], ec["th"][:], u2[:], OP.add)
            mk2 = et("mk2")
            vec.tensor_scalar(out=mk2[:], in0=ec["r"][:], scalar1=0.0,
                              scalar2=None, op0=OP.is_lt)
            u3 = et("u3")
            amul(u3[:], th2[:], -2.0, PI)
            u4 = et("u4")
            tt_(u4[:], mk2[:], u3[:], OP.mult)
            th3 = et("th3")
            tt_(th3[:], th2[:], u4[:], OP.add)
            phi = et("phi")
            amul(phi[:], th3[:], 1.0 / 3.0)
            ec["phi"] = phi
        for ec in ecs:
            et = ec["et"]
            c1, c3 = et("c1"), et("c3")
            nc.scalar.activation(c1[:], ec["phi"][:], AF.Sin,
                                 bias=bias_c[:, 0:1], scale=1.0)
            nc.scalar.activation(c3[:], ec["phi"][:], AF.Sin,
                                 bias=bias_c[:, 1:2], scale=1.0)
            ec["c1"], ec["c3"] = c1, c3
        out_r = out_ap.rearrange("(p t) k -> p t k", p=P)
        for ec in ecs:
            et = ec["et"]
            t0, TR, q, p2x = ec["t0"], ec["TR"], ec["q"], ec["p2x"]
            eigs = et("eigs", [P, TR, 3])
            g1, g2 = et("g1"), et("g2")
            tt_(g1[:], p2x[:], ec["c1"][:], OP.mult)
            tt_(eigs[:, :, 0], g1[:], q[:], OP.add)
            tt_(g2[:], p2x[:], ec["c3"][:], OP.mult)
            tt_(eigs[:, :, 2], q[:], g2[:], OP.subtract)
            q3 = et("q3")
            amul(q3[:], q[:], 3.0)
            tt_(q3[:], q3[:], eigs[:, :, 0], OP.subtract)
            tt_(eigs[:, :, 1], q3[:], eigs[:, :, 2], OP.subtract)
            nsp = min(4, TR)
            for d in range(nsp):
                sl = slice(t0 + d * TR // nsp, t0 + (d + 1) * TR // nsp)
                sle = slice(d * TR // nsp, (d + 1) * TR // nsp)
                nc.scalar.dma_start(out_r[:, sl, :], eigs[:, sle, :])

    # ---- main loop: skewed stage emission + spread eigen ----
    # emit oldest-tile stages first within each step (drain order) so no
    # engine queue holds younger work ahead of older dependencies
    stages = [(6, st_fixup), (5, st_matmuls), (4, st_mask), (3, st_sel),
              (1, st_neg), (0, st_squares)]
    chunks = [(0, T // 2), (T // 2, T - 6), (T - 6, T)]
    ecs = [{} for _ in chunks]
    gens = []
    for step in range(T + 6):
        for skew, fn in stages:
            tau = step - skew
            if 0 <= tau < T:
                fn(tau)
        for ci, (c0, c1_) in enumerate(chunks):
            if step == c1_ + 6:      # one step after fixup(c1-1)
                gens.append(emit_eigen_A(c0, c1_, ecs[ci]))
        for g in list(gens):
            try:
                next(g)
            except StopIteration:
                gens.remove(g)
    for g in gens:
        for _ in g:
            pass
    g3 = emit_eigen_A(*chunks[-1], ecs[-1]) if not ecs[-1] else None
    if g3 is not None:
        for _ in g3:
            pass
    emit_eigen_B(ecs)


def _emit_with_ident(ctx, tc, out_ap, uvt_ap, uvq_ap, xq_ap, M, jlo, jhi,
                     c0e, c1e):
    # identity matrix for the PE cov-transpose, built once
    nc = tc.nc
    const = ctx.enter_context(tc.tile_pool(name="identc", bufs=1))
    iota_a = const.tile([P, P], I32, tag="iota_a", name="iota_a")
    nc.gpsimd.iota(iota_a[:], pattern=[[1, P]], base=0, channel_multiplier=0)
    iota_b = const.tile([P, 1], I32, tag="iota_b", name="iota_b")
    nc.gpsimd.iota(iota_b[:], pattern=[[1, 1]], base=0, channel_multiplier=1)
    iota_af = const.tile([P, P], F32, tag="iota_af", name="iota_af")
    nc.gpsimd.tensor_copy(iota_af[:], iota_a[:])
    iota_bf = const.tile([P, 1], F32, tag="iota_bf", name="iota_bf")
    nc.gpsimd.tensor_copy(iota_bf[:], iota_b[:])
    ident = const.tile([P, P], F32, tag="ident", name="ident")
    nc.gpsimd.tensor_scalar(out=ident[:], in0=iota_af[:],
                            scalar1=iota_bf[:, 0:1],
                            scalar2=None, op0=OP.is_equal)
    _emit(ctx, tc, out_ap, uvt_ap, uvq_ap, xq_ap, M, ident, jlo, jhi,
          c0e, c1e)


def build_nc(M, jlo, jhi, c0e, c1e):
    nc = bacc.Bacc("TRN2", target_bir_lowering=False, debug=False,
                   enable_asserts=False)
    uvt_ap = nc.dram_tensor("uvt", (2, M), F32, kind="ExternalInput").ap()
    uvq_ap = nc.dram_tensor("uvq", (M, 2), F32, kind="ExternalInput").ap()
    xq_ap = nc.dram_tensor("xq", (M, 3), F32, kind="ExternalInput").ap()
    out_ap = nc.dram_tensor("out", (M, 3), F32, kind="ExternalOutput").ap()
    with tile.TileContext(nc) as tc:
        with ExitStack() as ctx:
            _emit_with_ident(ctx, tc, out_ap, uvt_ap, uvq_ap, xq_ap, M,
                             jlo, jhi, c0e, c1e)
    nc.compile()
    return nc


_NC_CACHE = {}


def _get_nc(M, jlo, jhi, c0e, c1e):
    key = (M, tuple(jlo), tuple(jhi), tuple(c0e), tuple(c1e))
    if key not in _NC_CACHE:
        _NC_CACHE[key] = build_nc(M, jlo, jhi, c0e, c1e)
    return _NC_CACHE[key]


def run(X, uv, trace: bool = False):
    B, M, _ = X.shape
    X = np.ascontiguousarray(X, dtype=np.float32)
    uv = np.ascontiguousarray(uv, dtype=np.float32)
    perms = []
    jlo = jhi = c0e = c1e = None
    for b in range(B):
        perm, jl, jh, ce0, ce1 = _batch_windows(uv[b])
        perms.append(perm)
        jlo = jl if jlo is None else np.minimum(jlo, jl)
        jhi = jh if jhi is None else np.maximum(jhi, jh)
        c0e = ce0 if c0e is None else np.minimum(c0e, ce0)
        c1e = ce1 if c1e is None else np.maximum(c1e, ce1)
    nc = _get_nc(M, jlo, jhi, c0e, c1e)
    T = M // P
    in_maps = []
    for b in range(B):
        us = uv[b][perms[b]]
        xs = X[b][perms[b]]
        in_maps.append({
            "uvt": np.ascontiguousarray(us.T),
            "uvq": np.ascontiguousarray(
                us.reshape(T, P, 2).transpose(1, 0, 2).reshape(M, 2)),
            "xq": np.ascontiguousarray(
                xs.reshape(T, P, 3).transpose(1, 0, 2).reshape(M, 3)),
        })
    res = run_bass_kernel_spmd(nc, in_maps, core_ids=list(range(B)),
                               trace=trace)
    out = np.empty((B, M, 3), np.float32)
    for b in range(B):
        o = res.results[b]["out"]
        out[b][perms[b]] = (o.reshape(P, T, 3).transpose(1, 0, 2)
                            .reshape(M, 3))
    return out, res


def kernel(X, uv):
    X = np.asarray(X)
    uv = np.asarray(uv)
    out, _ = run(X, uv, trace=False)
    return out.astype(np.float32)


# revision 44
# speedup vs baseline: 1.2252x; 1.0127x over previous
"""Trainium2 Bass kernel for DiffGeomPropsApprox (within-batch uv-space 16-NN
-> neighborhood covariance of X -> descending symmetric-3x3 eigenvalues).

Sharding: data-parallel over batch B=8, one batch per NeuronCore (8 cores).

Spatial windowing: host-side, each batch's points are sorted into a
serpentine strip order (strips = v-rank quantiles, u alternating
direction). A conservative per-query radius bound R(q) - refined to the
16th-smallest distance within a provably-sufficient cell box - yields,
per 128-query tile, a contiguous sorted-index window that contains every
query's exact 16-NN. The device only computes distances/top-k/mask/matmul
over that window (~5-7 of 32 tiles). Windows are computed at runtime from
the actual inputs (union across batches; one SPMD program).

Per-query-tile device pipeline (software-pipelined stage emission with
skews squares@0 / neg@+1 / sel@+3 / mask@+4 / matmul@+5 / fixup@+6, each
engine's in-order queue only sees work whose producers ran earlier):
  ACT:   squ=(u_c-u_q)^2, sqv=(v_c-v_q)^2 (Square, per-partition bias;
         exact reference f32 rounding)
  neg:   negdm = -(squ+sqv) exactly; production rotates GPSIMD-double /
         DVE fused custom op (LN_BWD_DX_ANT) / ACT-neg+GPSIMD-sub
  DVE:   max8 -> match_replace -> max8 over negdm -> -d16 (16th smallest)
  ACT:   mask = Sign(negdm + d16*(1+2^-22)) in {-1,+1} bf16
  DMA:   x-bar transpose mask -> [c-part, q]
  PE:    per window tile j: matmul acc += fbf_j^T wmask_j (18 features =
         9 x bf16 hi/lo) and ftot += fbf_j^T ones (separate PSUM groups)
  ACT:   fixup sum_sel = (acc+ftot)/2 via Identity bias-add; PE transpose;
         DVE slab copy.
Startup: partition-broadcast of u/v rows via PE fp32 ones-outer-product
(exact), PSUM-chunked; all input DMAs contiguous (host pre-transposes).
Eigenvalues: closed-form trig method in 3 chunks; sqrt phase (A) overlaps
the main loop via generator-spread emission, arctan/sin phase (B) runs
once at the end grouped by ACT table-set to avoid LUT reload thrash.
"""

from contextlib import ExitStack

import numpy as np

import concourse.bass as bass
import concourse.tile as tile
from concourse import bacc, mybir
from concourse.alu_op_type import AluOpType
from concourse.bass_utils import run_bass_kernel_spmd
from concourse.dve_ops import LN_BWD_DX_ANT

F32 = mybir.dt.float32
BF16 = mybir.dt.bfloat16
I32 = mybir.dt.int32
AF = mybir.ActivationFunctionType
OP = AluOpType

P = 128
K = 16
NEG_BIG = -3.0e38
PI = float(np.pi)
EPS_REL = 1.0 + 2.0 ** -22

G_STRIPS = 32          # equal-count strips (by v-rank)
H_INV = 256            # grid resolution for the coarse R(q) bound
H2_INV = 32            # cell-list resolution for the refined bound


# --------------------------- host-side windowing --------------------------- #

def _strip_perm(uv, G=G_STRIPS):
    """Serpentine strip order: strip = v-rank quantile, u asc/desc."""
    M = uv.shape[0]
    u, v = uv[:, 0], uv[:, 1]
    rank_v = np.empty(M, np.int64)
    rank_v[np.argsort(v, kind="stable")] = np.arange(M)
    strip = rank_v * G // M
    ukey = np.where(strip % 2 == 0, u, -u)
    return np.lexsort((ukey, strip))


def _geom_R(uv, h_inv=H_INV):
    """Coarse conservative bound on the 16-NN radius: smallest (2r+1)^2
    cell box centered on the point's cell holding >= K points; any K
    points in that box lie within (r+1)*h*sqrt(2)."""
    M = uv.shape[0]
    u, v = uv[:, 0], uv[:, 1]
    ci = np.minimum((u * h_inv).astype(np.int64), h_inv - 1)
    cj = np.minimum((v * h_inv).astype(np.int64), h_inv - 1)
    H = np.zeros((h_inv, h_inv), np.int64)
    np.add.at(H, (ci, cj), 1)
    S = np.zeros((h_inv + 1, h_inv + 1), np.int64)
    S[1:, 1:] = H.cumsum(0).cumsum(1)
    R = np.full(M, np.sqrt(2.0))
    done = np.zeros(M, bool)
    for rho in range(1, h_inv):
        i0 = np.clip(ci - rho, 0, h_inv); i1 = np.clip(ci + rho + 1, 0, h_inv)
        j0 = np.clip(cj - rho, 0, h_inv); j1 = np.clip(cj + rho + 1, 0, h_inv)
        cnt = S[i1, j1] - S[i0, j1] - S[i1, j0] + S[i0, j0]
        new = (~done) & (cnt >= K)
        R[new] = (rho + 1) / h_inv * np.sqrt(2.0)
        done |= new
        if done.all():
            break
    return R


def _refine_R(uv, R0, h_inv=H2_INV):
    """Tighten R to the 16th-smallest distance within the cell box that
    provably covers disc(q, R0).  disc(q, R0) holds >= K points (by R0's
    construction), the box covers the disc, so the 16th-nearest within
    the box is <= R0 and >= the true 16-NN radius."""
    M = uv.shape[0]
    u, v = uv[:, 0].astype(np.float64), uv[:, 1].astype(np.float64)
    ci = np.minimum((u * h_inv).astype(np.int64), h_inv - 1)
    cj = np.minimum((v * h_inv).astype(np.int64), h_inv - 1)
    cell = ci * h_inv + cj
    order = np.argsort(cell, kind="stable")
    csort = cell[order]
    ncell = h_inv * h_inv
    starts = np.searchsorted(csort, np.arange(ncell + 1))
    cnts = np.diff(starts)
    cmax = int(cnts.max())
    C = np.full((ncell, cmax), -1, np.int64)
    for c in range(ncell):
        C[c, :cnts[c]] = order[starts[c]:starts[c + 1]]
    rho = np.ceil(R0 * h_inv).astype(np.int64)
    R1 = np.empty(M, np.float64)
    for rv in np.unique(rho):
        sel = np.where(rho == rv)[0]
        offs = [(di, dj) for di in range(-rv, rv + 1)
                for dj in range(-rv, rv + 1)]
        cand = np.empty((len(sel), len(offs) * cmax), np.int64)
        for k, (di, dj) in enumerate(offs):
            ii = np.clip(ci[sel] + di, 0, h_inv - 1)
            jj = np.clip(cj[sel] + dj, 0, h_inv - 1)
            # out-of-range clamps may duplicate cells; harmless (extra
            # candidates only shrink the 16th distance toward truth; they
            # are real points so the bound stays valid)
            cand[:, k * cmax:(k + 1) * cmax] = C[ii * h_inv + jj]
        pad = cand < 0
        cid = np.where(pad, 0, cand)
        d2 = (u[sel, None] - u[cid]) ** 2 + (v[sel, None] - v[cid]) ** 2
        d2[pad] = np.inf
        # dedupe isn't needed for validity, but clamp-duplicated points
        # could make the 16th-smallest too small -> not conservative.
        # Sort candidate ids per row and inf-out repeats.
        si = np.argsort(cand, axis=1, kind="stable")
        cs = np.take_along_axis(cand, si, 1)
        dup = np.zeros_like(pad)
        dup[:, 1:] = cs[:, 1:] == cs[:, :-1]
        ds = np.take_along_axis(d2, si, 1)
        ds[dup | (cs < 0)] = np.inf
        R1[sel] = np.sqrt(np.partition(ds, K - 1, axis=1)[:, K - 1])
    return np.minimum(R0, np.nextafter(R1.astype(np.float32),
                                       np.float32(np.inf)))


def _batch_windows(uv, G=G_STRIPS):
    """perm + per-tile [jlo, jhi] (tile units) windows containing, for
    every query in the tile, all points within R(q) (hence its 16-NN)."""
    M = uv.shape[0]
    T = M // P
    spts = M // G
    perm = _strip_perm(uv, G)
    us, vs = uv[perm, 0], uv[perm, 1]
    R = _refine_R(uv, _geom_R(uv))
    Rq = R[perm]
    vmin = vs.reshape(G, spts).min(1)
    vmax = vs.reshape(G, spts).max(1)
    slo = np.clip(np.searchsorted(vmax, vs - Rq, side="left"), 0, G - 1)
    shi = np.clip(np.searchsorted(vmin, vs + Rq, side="right") - 1, 0, G - 1)
    plo = np.empty(M, np.int64)
    phi = np.empty(M, np.int64)
    for s in range(G):
        base = s * spts
        su = us[base:base + spts]
        asc = (s % 2 == 0)
        sua = su if asc else su[::-1]
        for sel, is_lo in ((slo == s, True), (shi == s, False)):
            if not sel.any():
                continue
            ulo = us[sel] - Rq[sel]
            uhi = us[sel] + Rq[sel]
            if is_lo:
                off = (np.searchsorted(sua, ulo, side="left") if asc else
                       spts - np.searchsorted(sua, uhi, side="right"))
                plo[sel] = base + np.clip(off, 0, spts - 1)
            else:
                off = (np.searchsorted(sua, uhi, side="right") - 1 if asc else
                       spts - 1 - np.searchsorted(sua, ulo, side="left"))
                phi[sel] = base + np.clip(off, 0, spts - 1)
    phi = np.maximum(phi, plo)
    jlo = np.empty(T, np.int64)
    jhi = np.empty(T, np.int64)
    c0e = np.empty(T, np.int64)
    c1e = np.empty(T, np.int64)
    for t in range(T):
        c0e[t] = plo[t * P:(t + 1) * P].min()
        c1e[t] = phi[t * P:(t + 1) * P].max() + 1
        jlo[t] = c0e[t] // P
        jhi[t] = (c1e[t] - 1) // P
    return perm, jlo, jhi, c0e, c1e


# ----------------------------- device kernel ------------------------------- #

def _emit(ctx, tc, out_ap, uvt_ap, uvq_ap, xq_ap, M, ident18, jlo, jhi,
          c0e, c1e):
    nc = tc.nc
    T = M // P
    NF = 18
    WTS = [int(jhi[t] - jlo[t] + 1) for t in range(T)]
    WMAX = max(WTS) * P
    # column-exact sub-ranges within each tile window (8-aligned) for the
    # pointwise passes; the pad columns only exist in the mask (set to -1)
    A0 = [(int(c0e[t]) - int(jlo[t]) * P) // 8 * 8 for t in range(T)]
    A1 = [min(-(-(int(c1e[t]) - int(jlo[t]) * P) // 8) * 8, WTS[t] * P)
          for t in range(T)]

    const = ctx.enter_context(tc.tile_pool(name="const", bufs=1))
    work = ctx.enter_context(tc.tile_pool(name="work", bufs=2))
    small = ctx.enter_context(tc.tile_pool(name="small", bufs=14))
    psum = ctx.enter_context(tc.tile_pool(name="psum", bufs=2, space="PSUM"))
    epool = ctx.enter_context(tc.tile_pool(name="eig", bufs=1))

    # ---- broadcast candidate coords across partitions ----
    # uvt is host-transposed [2, M]: step-0 loads are contiguous rows.
    # Doubling chain, column-split across two queues per coordinate.
    # slab layouts [P, T, k]: host pre-transposed so loads are contiguous
    uv_slab = const.tile([P, T, 2], F32, tag="uv_slab")
    uv_r = uvq_ap.rearrange("(p t) k -> p t k", p=P)
    x_slab = const.tile([P, T, 3], F32, tag="x_slab")
    x_r = xq_ap.rearrange("(p t) k -> p t k", p=P)
    nc.sync.dma_start(uv_slab[:], uv_r[:])
    nc.scalar.dma_start(x_slab[:], x_r[:])

    # partition-broadcast of u/v rows via PE fp32 outer product with a ones
    # column (exact: 1.0*x). 512-col PSUM chunks, copied out on ACT/DVE.
    u_b = const.tile([P, M], F32, tag="u_b")
    v_b = const.tile([P, M], F32, tag="v_b")
    ones1 = const.tile([1, P], F32, tag="ones1")
    nc.gpsimd.memset(ones1[:], 1.0)
    urow = const.tile([1, M], F32, tag="urow")
    vrow = const.tile([1, M], F32, tag="vrow")
    nc.sync.dma_start(urow[:], uvt_ap[0:1, :])
    nc.scalar.dma_start(vrow[:], uvt_ap[1:2, :])
    BC = 512
    for ci, c in enumerate(range(0, M, BC)):
        for row, dst in ((urow, u_b), (vrow, v_b)):
            pb = psum.tile([P, BC], F32, tag="bc", name="bc", bufs=2)
            nc.tensor.matmul(pb[:], lhsT=ones1[:], rhs=row[0:1, c:c + BC],
                             start=True, stop=True)
            if (ci + (0 if dst is u_b else 1)) % 2 == 0:
                nc.vector.tensor_copy(dst[:, c:c + BC], pb[:])
            else:
                nc.scalar.copy(dst[:, c:c + BC], pb[:])
    nuv = const.tile([P, T, 2], F32, tag="nuv")
    nc.vector.tensor_scalar(out=nuv[:], in0=uv_slab[:], scalar1=-1.0,
                            scalar2=None, op0=OP.mult)

    # ---- features: [x y z x2 y2 z2 xy xz yz] as bf16 hi/lo ----
    pairs = [(0, 0), (1, 1), (2, 2), (0, 1), (0, 2), (1, 2)]
    fsl = work.tile([P, T, 9], F32, tag="fsl", name="fsl", bufs=1)
    nc.vector.tensor_copy(fsl[:, :, 0:3], x_slab[:])
    for i, (a, b) in enumerate(pairs):
        nc.vector.tensor_tensor(out=fsl[:, :, 3 + i], in0=x_slab[:, :, a],
                                in1=x_slab[:, :, b], op=OP.mult)
    fbf = const.tile([P, T, NF], BF16, tag="fbf")
    nc.vector.tensor_copy(fbf[:, :, 0:9], fsl[:])
    fhi32 = work.tile([P, T, 9], F32, tag="fhi32", name="fhi32", bufs=1)
    nc.vector.tensor_copy(fhi32[:], fbf[:, :, 0:9])
    nc.vector.tensor_tensor(out=fbf[:, :, 9:18], in0=fsl[:], in1=fhi32[:],
                            op=OP.subtract)

    ones_c = const.tile([P, 1], BF16, tag="ones_c")
    nc.gpsimd.memset(ones_c[:], 1.0)
    zeros = const.tile([P, WMAX], F32, tag="zeros")
    nc.gpsimd.memset(zeros[:], 0.0)
    bias_c = const.tile([P, 2], F32, tag="bias_c")
    nc.gpsimd.memset(bias_c[:, 0:1], PI / 2)
    nc.gpsimd.memset(bias_c[:, 1:2], PI / 6)

    cov = const.tile([P, T, NF], F32, tag="cov")

    # ---- pipeline stages -------------------------------------------------
    state = {}

    def st_squares(t):
        c0 = int(jlo[t]) * P
        a0, a1 = A0[t], A1[t]
        squ = work.tile([P, WMAX], F32, tag="sq", name="squ", bufs=8)
        nc.scalar.activation(squ[:, a0:a1], u_b[:, c0 + a0:c0 + a1],
                             AF.Square, bias=nuv[:, t, 0:1], scale=1.0)
        sqv = work.tile([P, WMAX], F32, tag="sq", name="sqv", bufs=8)
        nc.scalar.activation(sqv[:, a0:a1], v_b[:, c0 + a0:c0 + a1],
                             AF.Square, bias=nuv[:, t, 1:2], scale=1.0)
        state[t] = {"squ": squ, "sqv": sqv, "w": WTS[t] * P}

    def st_neg(t):
        # negdm = -(squ + sqv), exact; production rotates across engines:
        #   t%3==0: GPSIMD (0-squ) then (nsq-sqv)
        #   t%3==1: DVE fused custom op (squ - sqv*-1 - 0) * -1
        #   t%3==2: ACT -squ, then GPSIMD (nsq - sqv)
        s = state[t]
        a0, a1 = A0[t], A1[t]
        negdm = work.tile([P, WMAX], F32, tag="negdm", name="negdm", bufs=6)
        r = t % 3
        if r == 1:
            nc.vector._custom_dve(LN_BWD_DX_ANT, out=negdm[:, a0:a1],
                                  in0=s["squ"][:, a0:a1],
                                  in1=s["sqv"][:, a0:a1],
                                  s0=-1.0, s1=0.0, imm2=-1.0)
        else:
            nsq = work.tile([P, WMAX], F32, tag="dm", name="nsq", bufs=4)
            if r == 0:
                nc.gpsimd.tensor_tensor(out=nsq[:, a0:a1],
                                        in0=zeros[:, a0:a1],
                                        in1=s["squ"][:, a0:a1],
                                        op=OP.subtract)
            else:
                nc.scalar.activation(nsq[:, a0:a1], s["squ"][:, a0:a1],
                                     AF.Copy, bias=0.0, scale=-1.0)
            nc.gpsimd.tensor_tensor(out=negdm[:, a0:a1], in0=nsq[:, a0:a1],
                                    in1=s["sqv"][:, a0:a1], op=OP.subtract)
        s["negdm"] = negdm

    def st_sel(t):
        s = state[t]
        a0, a1 = A0[t], A1[t]
        negdm = s["negdm"]
        m1 = small.tile([P, 8], F32, tag="m1", name="m1")
        nc.vector.max(m1[:], negdm[:, a0:a1])
        mr = work.tile([P, WMAX], F32, tag="mr", name="mr", bufs=3)
        nc.vector.match_replace(mr[:, a0:a1], m1[:], negdm[:, a0:a1],
                                NEG_BIG)
        m2 = small.tile([P, 8], F32, tag="m2", name="m2")
        nc.vector.max(m2[:], mr[:, a0:a1])
        # Sign-mask bias: +d16*(1+2^-22)  (m2[7] = -d16)
        nt16p = small.tile([P, 1], F32, tag="nt16p", name="nt16p")
        nc.vector.tensor_scalar(out=nt16p[:], in0=m2[:, 7:8],
                                scalar1=-EPS_REL, scalar2=None, op0=OP.mult)
        s["nt16p"] = nt16p

    def st_mask(t):
        s = state[t]
        w = s["w"]
        a0, a1 = A0[t], A1[t]
        # {-1,+1} mask in bf16 on ACT (Sign LUT); +1 iff d <= d16*(1+eps).
        # Pad columns outside [a0,a1) hold -1 (unselected) so the ftot
        # correction stays consistent over the full tile window.
        wmask = work.tile([P, WMAX], BF16, tag="wmask", name="wmask", bufs=3)
        if a0 > 0:
            nc.vector.memset(wmask[:, 0:a0], -1.0)
        if a1 < w:
            nc.vector.memset(wmask[:, a1:w], -1.0)
        nc.scalar.activation(wmask[:, a0:a1], s["negdm"][:, a0:a1], AF.Sign,
                             bias=s["nt16p"][:], scale=1.0)
        wt = work.tile([P, WMAX // P, P], BF16, tag="wt", name="wt", bufs=4)
        nc.sync.dma_start(wt[:, 0:w // P, :], wmask[:, 0:w], transpose=True)
        s["wt"] = wt

    def st_matmuls(t):
        s = state[t]
        w = s["w"]
        wt = s["wt"]
        # acc and the window feature-total share one PSUM bank: [:, 0:P]
        # accumulates fbf^T wmask, [:, P] accumulates fbf^T ones
        acc = psum.tile([NF, P + 1], F32, tag="acc", name="acc", bufs=3)
        j0 = int(jlo[t])
        nj = w // P
        for jl in range(nj):
            nc.tensor.matmul(acc[:, 0:P], lhsT=fbf[:, j0 + jl, :],
                             rhs=wt[:, jl, :], start=(jl == 0),
                             stop=(jl == nj - 1))
        for jl in range(nj):
            nc.tensor.matmul(acc[:, P:P + 1], lhsT=fbf[:, j0 + jl, :],
                             rhs=ones_c[:], start=(jl == 0),
                             stop=(jl == nj - 1))
        s["acc"] = acc

    def st_fixup(t):
        s = state.pop(t)
        # sum_sel = (acc + ftot)/2 : ftot PSUM->SBUF (scaled), then
        # Identity with per-partition bias (both on ACT, close to PSUM)
        ftoth = small.tile([NF, 1], F32, tag="ftoth", name="ftoth")
        nc.vector.tensor_scalar(out=ftoth[:], in0=s["acc"][:, P:P + 1],
                                scalar1=0.5, scalar2=None, op0=OP.mult)
        covg = work.tile([NF, P], F32, tag="covg", name="covg", bufs=2)
        nc.scalar.activation(covg[:], s["acc"][:, 0:P], AF.Identity,
                             bias=ftoth[:], scale=0.5)
        ctp = psum.tile([P, NF], F32, tag="ctp", name="ctp", bufs=3)
        nc.tensor.matmul(ctp[:], lhsT=covg[:], rhs=ident18[0:NF, 0:NF],
                         is_transpose=True)
        nc.vector.tensor_copy(cov[:, t, :], ctp[:])

    # ---- eigen phase, split into A (through arctan input) and B
    # (arctan onward), emitted as generators so ops spread across steps.
    # Grouping all Sqrt work (A) apart from Arctan/Sin work (B) avoids ACT
    # table-set thrashing; B runs once for all chunks at the end.
    vec = nc.vector

    def tt_(out, a, b, op):
        vec.tensor_tensor(out=out, in0=a, in1=b, op=op)

    def tg_(out, a, b, op):
        # independent (off-critical-chain) eigen products go to GPSIMD
        nc.gpsimd.tensor_tensor(out=out, in0=a, in1=b, op=op)

    def amul(out, a, scale, bias=0.0):
        nc.scalar.activation(out, a, AF.Copy, bias=bias, scale=scale)

    def emit_eigen_A(t0, t1, ec):
        TR = t1 - t0
        covh = cov[:, t0:t1, :]

        def et(name, shape=None):
            return epool.tile(shape or [P, TR], F32, tag=f"{name}_{t0}",
                              name=f"{name}_{t0}")

        ec.update(t0=t0, TR=TR, et=et)
        S = et("S", [P, TR, 9])
        tt_(S[:], covh[:, :, 0:9], covh[:, :, 9:18], OP.add)
        Sq = et("Sq", [P, TR, 3])
        amul(Sq[:], S[:, :, 0:3], 0.25)
        yield
        cm = et("cm", [P, TR, 6])
        tmps = [et(f"cmt{i}") for i in range(6)]
        for i, (a, b) in enumerate(pairs):
            tg_(tmps[i][:], Sq[:, :, a], Sq[:, :, b], OP.mult)
        yield
        for i in range(6):
            tt_(cm[:, :, i], S[:, :, 3 + i], tmps[i][:], OP.subtract)
        yield
        cxx, cyy, czz = cm[:, :, 0], cm[:, :, 1], cm[:, :, 2]
        cxy, cxz, cyz = cm[:, :, 3], cm[:, :, 4], cm[:, :, 5]

        q = et("q")
        q1 = et("q1")
        tt_(q1[:], cxx, cyy, OP.add)
        tt_(q1[:], q1[:], czz, OP.add)
        amul(q[:], q1[:], 1.0 / 3.0)
        b00, b11, b22 = et("b00"), et("b11"), et("b22")
        tt_(b00[:], cxx, q[:], OP.subtract)
        tt_(b11[:], cyy, q[:], OP.subtract)
        tt_(b22[:], czz, q[:], OP.subtract)
        yield
        # p2 = b00^2+b11^2+b22^2 + 2(cxy^2+cxz^2+cyz^2)
        pa, pb, pc_ = et("pa"), et("pb"), et("pc2")
        oa, ob, oc = et("oa"), et("ob"), et("oc")
        tg_(pa[:], b00[:], b00[:], OP.mult)
        tg_(pb[:], b11[:], b11[:], OP.mult)
        tt_(pc_[:], b22[:], b22[:], OP.mult)
        tg_(oa[:], cxy, cxy, OP.mult)
        tg_(ob[:], cxz, cxz, OP.mult)
        tt_(oc[:], cyz, cyz, OP.mult)
        yield
        p2 = et("p2")
        s1, s3 = et("s1"), et("s3")
        tt_(s1[:], pa[:], pb[:], OP.add)
        tt_(s1[:], s1[:], pc_[:], OP.add)
        tt_(s3[:], oa[:], ob[:], OP.add)
        tt_(s3[:], s3[:], oc[:], OP.add)
        s5 = et("s5")
        amul(s5[:], s3[:], 2.0)
        tt_(p2[:], s1[:], s5[:], OP.add)
        p = et("p")
        nc.scalar.activation(p[:], p2[:], AF.Sqrt, bias=0.0, scale=1.0 / 6.0)
        yield
        pc = et("pc")
        vec.tensor_scalar(out=pc[:], in0=p[:], scalar1=1e-30, scalar2=None,
                          op0=OP.max)
        ip = et("ip")
        vec.reciprocal(ip[:], pc[:])
        p2x = et("p2x")
        amul(p2x[:], p[:], 2.0)
        # det(A - qI)
        d1, d3, d4 = et("d1"), et("d3"), et("d4")
        tt_(d1[:], b11[:], b22[:], OP.mult)
        tt_(d3[:], d1[:], oc[:], OP.subtract)
        tt_(d4[:], b00[:], d3[:], OP.mult)
        e1, e2, e3, e4 = et("e1"), et("e2"), et("e3"), et("e4")
        tg_(e1[:], cxy, b22[:], OP.mult)
        tg_(e2[:], cyz, cxz, OP.mult)
        tt_(e3[:], e1[:], e2[:], OP.subtract)
        tt_(e4[:], cxy, e3[:], OP.mult)
        yield
        f1, f2, f3, f4 = et("f1"), et("f2"), et("f3"), et("f4")
        tg_(f1[:], cxy, cyz, OP.mult)
        tg_(f2[:], b11[:], cxz, OP.mult)
        tt_(f3[:], f1[:], f2[:], OP.subtract)
        tt_(f4[:], cxz, f3[:], OP.mult)
        det = et("det")
        tt_(det[:], d4[:], e4[:], OP.subtract)
        tt_(det[:], det[:], f4[:], OP.add)
        yield
        # r = clamp(det * ip^3 / 2, -1, 1)
        i2, i3 = et("i2"), et("i3")
        tt_(i2[:], ip[:], ip[:], OP.mult)
        tt_(i3[:], i2[:], ip[:], OP.mult)
        r = et("r")
        tt_(r[:], det[:], i3[:], OP.mult)
        vec.tensor_scalar(out=r[:], in0=r[:], scalar1=0.5, scalar2=1.0,
                          op0=OP.mult, op1=OP.min)
        vec.tensor_scalar(out=r[:], in0=r[:], scalar1=-1.0, scalar2=None,
                          op0=OP.max)
        yield
        # acos(r) via octant-reduced arctan: A ends at the arctan INPUT
        rr = et("rr")
        tt_(rr[:], r[:], r[:], OP.mult)
        aab = et("aab")
        nc.scalar.activation(aab[:], rr[:], AF.Sqrt, bias=0.0, scale=1.0)
        rr2 = et("rr2")
        vec.tensor_scalar(out=rr2[:], in0=rr[:], scalar1=-1.0, scalar2=1.0,
                          op0=OP.mult, op1=OP.add)
        s = et("s")
        nc.scalar.activation(s[:], rr2[:], AF.Sqrt, bias=0.0, scale=1.0)
        mn, mx = et("mn"), et("mx")
        tt_(mn[:], aab[:], s[:], OP.min)
        tt_(mx[:], aab[:], s[:], OP.max)
        imx = et("imx")
        vec.reciprocal(imx[:], mx[:])
        ratio = et("ratio")
        tt_(ratio[:], mn[:], imx[:], OP.mult)
        ec.update(q=q, p2x=p2x, r=r, s=s, aab=aab, ratio=ratio)

    def emit_eigen_B(ecs):
        # all arctans first, then the (table-free) fold chains, then all
        # sins, then eigenvalue assembly + output DMA per chunk
        for ec in ecs:
            th = ec["et"]("th")
            nc.scalar.activation(th[:], ec["ratio"][:], AF.Arctan, bias=0.0,
                                 scale=1.0)
            ec["th"] = th
        for ec in ecs:
            et = ec["et"]
            mk = et("mk")
            tt_(mk[:], ec["s"][:], ec["aab"][:], OP.is_gt)
            u1 = et("u1")
            amul(u1[:], ec["th"][:], -2.0, PI / 2)
            u2 = et("u2")
            tt_(u2[:], mk[:], u1[:], OP.mult)
            th2 = et("th2")
            tt_(th2[:], ec["th"][:], u2[:], OP.add)
            mk2 = et("mk2")
            vec.tensor_scalar(out=mk2[:], in0=ec["r"][:], scalar1=0.0,
                              scalar2=None, op0=OP.is_lt)
            u3 = et("u3")
            amul(u3[:], th2[:], -2.0, PI)
            u4 = et("u4")
            tt_(u4[:], mk2[:], u3[:], OP.mult)
            th3 = et("th3")
            tt_(th3[:], th2[:], u4[:], OP.add)
            phi = et("phi")
            amul(phi[:], th3[:], 1.0 / 3.0)
            ec["phi"] = phi
        for ec in ecs:
            et = ec["et"]
            c1, c3 = et("c1"), et("c3")
            nc.scalar.activation(c1[:], ec["phi"][:], AF.Sin,
                                 bias=bias_c[:, 0:1], scale=1.0)
            nc.scalar.activation(c3[:], ec["phi"][:], AF.Sin,
                                 bias=bias_c[:, 1:2], scale=1.0)
            ec["c1"], ec["c3"] = c1, c3
        out_r = out_ap.rearrange("(p t) k -> p t k", p=P)
        for ec in ecs:
            et = ec["et"]
            t0, TR, q, p2x = ec["t0"], ec["TR"], ec["q"], ec["p2x"]
            eigs = et("eigs", [P, TR, 3])
            g1, g2 = et("g1"), et("g2")
            tt_(g1[:], p2x[:], ec["c1"][:], OP.mult)
            tt_(eigs[:, :, 0], g1[:], q[:], OP.add)
            tt_(g2[:], p2x[:], ec["c3"][:], OP.mult)
            tt_(eigs[:, :, 2], q[:], g2[:], OP.subtract)
            q3 = et("q3")
            amul(q3[:], q[:], 3.0)
            tt_(q3[:], q3[:], eigs[:, :, 0], OP.subtract)
            tt_(eigs[:, :, 1], q3[:], eigs[:, :, 2], OP.subtract)
            nsp = min(4, TR)
            for d in range(nsp):
                sl = slice(t0 + d * TR // nsp, t0 + (d + 1) * TR // nsp)
                sle = slice(d * TR // nsp, (d + 1) * TR // nsp)
                nc.scalar.dma_start(out_r[:, sl, :], eigs[:, sle, :])

    # ---- main loop: skewed stage emission + spread eigen ----
    # emit oldest-tile stages first within each step (drain order) so no
    # engine queue holds younger work ahead of older dependencies
    stages = [(6, st_fixup), (5, st_matmuls), (4, st_mask), (3, st_sel),
              (1, st_neg), (0, st_squares)]
    chunks = [(0, T // 2), (T // 2, T - 8), (T - 8, T)]
    ecs = [{} for _ in chunks]
    gens = []
    for step in range(T + 6):
        for skew, fn in stages:
            tau = step - skew
            if 0 <= tau < T:
                fn(tau)
        for ci, (c0, c1_) in enumerate(chunks):
            if step == c1_ + 6:      # one step after fixup(c1-1)
                gens.append(emit_eigen_A(c0, c1_, ecs[ci]))
        for g in list(gens):
            try:
                next(g)
            except StopIteration:
                gens.remove(g)
    for g in gens:
        for _ in g:
            pass
    g3 = emit_eigen_A(*chunks[-1], ecs[-1]) if not ecs[-1] else None
    if g3 is not None:
        for _ in g3:
            pass
    emit_eigen_B(ecs)


def _emit_with_ident(ctx, tc, out_ap, uvt_ap, uvq_ap, xq_ap, M, jlo, jhi,
                     c0e, c1e):
    # identity matrix for the PE cov-transpose, built once
    nc = tc.nc
    const = ctx.enter_context(tc.tile_pool(name="identc", bufs=1))
    iota_a = const.tile([P, P], I32, tag="iota_a", name="iota_a")
    nc.gpsimd.iota(iota_a[:], pattern=[[1, P]], base=0, channel_multiplier=0)
    iota_b = const.tile([P, 1], I32, tag="iota_b", name="iota_b")
    nc.gpsimd.iota(iota_b[:], pattern=[[1, 1]], base=0, channel_multiplier=1)
    iota_af = const.tile([P, P], F32, tag="iota_af", name="iota_af")
    nc.gpsimd.tensor_copy(iota_af[:], iota_a[:])
    iota_bf = const.tile([P, 1], F32, tag="iota_bf", name="iota_bf")
    nc.gpsimd.tensor_copy(iota_bf[:], iota_b[:])
    ident = const.tile([P, P], F32, tag="ident", name="ident")
    nc.gpsimd.tensor_scalar(out=ident[:], in0=iota_af[:],
                            scalar1=iota_bf[:, 0:1],
                            scalar2=None, op0=OP.is_equal)
    _emit(ctx, tc, out_ap, uvt_ap, uvq_ap, xq_ap, M, ident, jlo, jhi,
          c0e, c1e)


def build_nc(M, jlo, jhi, c0e, c1e):
    nc = bacc.Bacc("TRN2", target_bir_lowering=False, debug=False,
                   enable_asserts=False)
    uvt_ap = nc.dram_tensor("uvt", (2, M), F32, kind="ExternalInput").ap()
    uvq_ap = nc.dram_tensor("uvq", (M, 2), F32, kind="ExternalInput").ap()
    xq_ap = nc.dram_tensor("xq", (M, 3), F32, kind="ExternalInput").ap()
    out_ap = nc.dram_tensor("out", (M, 3), F32, kind="ExternalOutput").ap()
    with tile.TileContext(nc) as tc:
        with ExitStack() as ctx:
            _emit_with_ident(ctx, tc, out_ap, uvt_ap, uvq_ap, xq_ap, M,
                             jlo, jhi, c0e, c1e)
    nc.compile()
    return nc


_NC_CACHE = {}


def _get_nc(M, jlo, jhi, c0e, c1e):
    key = (M, tuple(jlo), tuple(jhi), tuple(c0e), tuple(c1e))
    if key not in _NC_CACHE:
        _NC_CACHE[key] = build_nc(M, jlo, jhi, c0e, c1e)
    return _NC_CACHE[key]


def run(X, uv, trace: bool = False):
    B, M, _ = X.shape
    X = np.ascontiguousarray(X, dtype=np.float32)
    uv = np.ascontiguousarray(uv, dtype=np.float32)
    perms = []
    jlo = jhi = c0e = c1e = None
    for b in range(B):
        perm, jl, jh, ce0, ce1 = _batch_windows(uv[b])
        perms.append(perm)
        jlo = jl if jlo is None else np.minimum(jlo, jl)
        jhi = jh if jhi is None else np.maximum(jhi, jh)
        c0e = ce0 if c0e is None else np.minimum(c0e, ce0)
        c1e = ce1 if c1e is None else np.maximum(c1e, ce1)
    nc = _get_nc(M, jlo, jhi, c0e, c1e)
    T = M // P
    in_maps = []
    for b in range(B):
        us = uv[b][perms[b]]
        xs = X[b][perms[b]]
        in_maps.append({
            "uvt": np.ascontiguousarray(us.T),
            "uvq": np.ascontiguousarray(
                us.reshape(T, P, 2).transpose(1, 0, 2).reshape(M, 2)),
            "xq": np.ascontiguousarray(
                xs.reshape(T, P, 3).transpose(1, 0, 2).reshape(M, 3)),
        })
    res = run_bass_kernel_spmd(nc, in_maps, core_ids=list(range(B)),
                               trace=trace)
    out = np.empty((B, M, 3), np.float32)
    for b in range(B):
        o = res.results[b]["out"]
        out[b][perms[b]] = (o.reshape(P, T, 3).transpose(1, 0, 2)
                            .reshape(M, 3))
    return out, res


def kernel(X, uv):
    X = np.asarray(X)
    uv = np.asarray(uv)
    out, _ = run(X, uv, trace=False)
    return out.astype(np.float32)
